# revision 1
# baseline (speedup 1.0000x reference)
"""Trainium2 Bass kernel for the CNNFusing ragged-session attention pooling module.

Computes, per session s over its contiguous token range:
    v_mean   = mean(hidden[s])                                  [H]
    pos_h[t] = tanh(hidden[t] @ Wp1 + (pos_table @ Wp2 + b_pos)[rp[t]])
    gate[t]  = sigmoid(v_mean @ W1 + b1 + pos_h[t] @ W2 + b2)
    alpha[t] = gate[t] @ qw + qb
    h_s      = sum_t alpha[t] * hidden[t]                       [B, H]

Strategy: pure data parallelism over sessions across 8 cores. Each core's
sessions are packed into fixed 512-token chunks (sessions never straddle a
chunk). All ragged ops (segment sum, per-token broadcast of session values,
position-table gather) become one-hot matmuls on the PE array. Operands are
fp16 (fp32 PSUM accumulation); sigmoid is folded into tanh so ScalarE keeps a
single activation table.
"""

import numpy as np

import concourse.bass as bass
import concourse.mybir as mybir
import concourse.tile as tile
from concourse.vector_clock import ScopedClock
from concourse.bass_utils import run_bass_kernel_spmd

H = 256
TC = 512      # tokens per chunk
S = 32        # max sessions per chunk (observed max ~14 for this distribution)
KT = TC // 128  # 128-token k-tiles per chunk
N_CORES = 8

F16 = mybir.dt.float16
F32 = mybir.dt.float32


# --------------------------------------------------------------------------
# The walrus build here accepts only ONE sync-wait command per instruction,
# while Tile may attach several (tail drain, DMA transposes, ...). Hoist all
# but the last wait of such instructions onto standalone event-semaphore
# waits inserted just before them on the same engine (sequencer executes in
# order, so semantics are preserved).
_waitsplit_uid = [0]


def _split_multi_waits(nc):
    for fn in nc.m.functions:
        for bb in fn.blocks:
            insts = bb.instructions
            i = 0
            while i < len(insts):
                inst = insts[i]
                si = getattr(inst, "sync_info", None)
                waits = list(si.on_wait) if si is not None and si.on_wait else []
                if len(waits) > 1:
                    si.on_wait = waits[-1:]
                    for w in waits[:-1]:
                        ev = mybir.InstEventSemaphore(
                            name=f"I-waitsplit-{_waitsplit_uid[0]}", ins=[], outs=[]
                        )
                        _waitsplit_uid[0] += 1
                        ev.engine = inst.engine
                        ev.sync_info = mybir.SyncInfo(on_wait=[w], on_update=[])
                        insts.insert(i, ev)
                        i += 1
                i += 1
# --------------------------------------------------------------------------


def _plan(seq_len):
    """Assign contiguous sessions to cores (balanced tokens), then pack each
    core's sessions into chunks of <= TC tokens and <= S sessions."""
    lens = np.asarray(seq_len, dtype=np.int64)
    B = len(lens)
    cum = np.cumsum(lens)
    total = int(cum[-1])
    starts = cum - lens  # token start of each session

    bounds = [0]
    for i in range(1, N_CORES):
        bounds.append(int(np.searchsorted(cum, total * i / N_CORES)))
    bounds.append(B)

    core_chunks = []
    for c in range(N_CORES):
        lo, hi = bounds[c], bounds[c + 1]
        out = []
        s = lo
        while s < hi:
            e = s
            tok = 0
            while e < hi and e - s < S and tok + lens[e] <= TC:
                tok += int(lens[e])
                e += 1
            assert e > s, "single session longer than chunk"
            out.append((s, e))
            s = e
        core_chunks.append(out)
    C = max(len(x) for x in core_chunks)
    return lens, starts, core_chunks, C


def _pack_inputs(hidden, reverse_pos, pw16, lens, starts, core_chunks, C):
    xt16 = np.zeros((N_CORES, C, TC, H), np.float16)
    pft16 = np.zeros((N_CORES, C, TC, H), np.float16)
    seg_row = np.full((N_CORES, C, TC), -1.0, np.float16)
    recip = np.zeros((N_CORES, C, S), np.float32)

    out_core = np.zeros(len(lens), np.int32)
    out_chunk = np.zeros(len(lens), np.int32)
    out_local = np.zeros(len(lens), np.int32)

    hidden16 = hidden.astype(np.float16)
    rp = np.asarray(reverse_pos)
    for core, chs in enumerate(core_chunks):
        for ci, (s, e) in enumerate(chs):
            t0 = int(starts[s])
            ntok = int(lens[s:e].sum())
            ns = e - s
            xt16[core, ci, :ntok] = hidden16[t0 : t0 + ntok]
            pft16[core, ci, :ntok] = pw16[rp[t0 : t0 + ntok]]
            seg_row[core, ci, :ntok] = np.repeat(
                np.arange(ns, dtype=np.float16), lens[s:e]
            )
            recip[core, ci, :ns] = 1.0 / lens[s:e]
            out_core[s:e] = core
            out_chunk[s:e] = ci
            out_local[s:e] = np.arange(ns)

    # seg_col: [N, 128, C, KT] — per-token local session id, column layout
    seg_col = np.ascontiguousarray(
        seg_row.reshape(N_CORES, C, KT, 128).transpose(0, 3, 1, 2)
    ).astype(np.float32)
    # recip: [N, S, C]
    recip = np.ascontiguousarray(recip.transpose(0, 2, 1))
    return xt16, pft16, seg_row, seg_col, recip, (out_core, out_chunk, out_local)


def _pack_weights(pos_table, W_pos, b_pos, W1, b1, W2, b2, qw, qb):
    Wp = np.asarray(W_pos, np.float32)
    wp1 = Wp[:H]
    pwf = np.asarray(pos_table, np.float32) @ Wp[H:] + np.asarray(b_pos, np.float32)
    pw = np.zeros((H, H), np.float32)
    pw[: pwf.shape[0]] = pwf

    def pack_lhsT(M):  # [256, 256] -> [128, 2, 256] (c_in half-major)
        return (
            np.ascontiguousarray(
                M.reshape(2, 128, H).transpose(1, 0, 2)
            ).astype(np.float16)
        )

    wp1p = pack_lhsT(wp1)
    pw16 = pw.astype(np.float16)  # gathered per token on host into pos_feat
    w1p = pack_lhsT(np.asarray(W1, np.float32))
    w2p = pack_lhsT(np.asarray(W2, np.float32))

    qwf = np.asarray(qw, np.float32).reshape(H)
    # alpha = gate@qw + qb with gate = 0.5*gt + 0.5 folds to
    # alpha = 0.5*(gt@qw) + (qb + sum(qw)/2); the 0.5 is applied post-matmul.
    qwh = np.ascontiguousarray(qwf.reshape(2, 128).T).astype(np.float16)
    qbp = float(np.asarray(qb, np.float32).reshape(()) + qwf.sum() / 2.0)
    bcf = 0.5 * (np.asarray(b1, np.float32) + np.asarray(b2, np.float32))
    bch = np.ascontiguousarray(bcf.reshape(2, 128).T).astype(np.float32)

    iota_at = np.broadcast_to(
        np.arange(S, dtype=np.float16), (128, S)
    ).copy()
    iota_s = np.arange(S, dtype=np.float32).reshape(S, 1)
    ident = np.eye(128, dtype=np.float16)
    return dict(
        wp1=wp1p, w1=w1p, w2=w2p, qwh=qwh, bch=bch,
        iota_at=iota_at, iota_s=iota_s, ident=ident,
    ), qbp, pw16


def _build_bass(C, qbp):
    nc = bass.Bass("TRN2", target_bir_lowering=False, debug=False,
                   num_devices=N_CORES)

    xt = nc.dram_tensor("xt", [C, TC, H], F16, kind="ExternalInput")
    pft = nc.dram_tensor("pft", [C, TC, H], F16, kind="ExternalInput")
    seg_row = nc.dram_tensor("seg_row", [C, TC], F16, kind="ExternalInput")
    seg_col = nc.dram_tensor("seg_col", [128, C, KT], F32, kind="ExternalInput")
    recip = nc.dram_tensor("recip", [S, C], F32, kind="ExternalInput")
    wp1 = nc.dram_tensor("wp1", [128, 2, H], F16, kind="ExternalInput")
    w1 = nc.dram_tensor("w1", [128, 2, H], F16, kind="ExternalInput")
    w2 = nc.dram_tensor("w2", [128, 2, H], F16, kind="ExternalInput")
    qwh = nc.dram_tensor("qwh", [128, 2], F16, kind="ExternalInput")
    bch = nc.dram_tensor("bch", [128, 2], F32, kind="ExternalInput")
    iota_at = nc.dram_tensor("iota_at", [128, S], F16, kind="ExternalInput")
    iota_s = nc.dram_tensor("iota_s", [S, 1], F32, kind="ExternalInput")
    ident = nc.dram_tensor("ident", [128, 128], F16, kind="ExternalInput")
    hs = nc.dram_tensor("hs", [C, S, H], F32, kind="ExternalOutput")

    eq = mybir.AluOpType.is_equal
    mult = mybir.AluOpType.mult
    add = mybir.AluOpType.add
    Tanh = mybir.ActivationFunctionType.Tanh
    GRP = 4  # chunks per broadcast DMA

    with tile.TileContext(nc) as tc:
        with (
            tc.tile_pool(name="consts", bufs=1) as pc,
            tc.tile_pool(name="work", bufs=6) as pwk,
            # PSUM banks: ga 2x1 + ph 1x2 + gate 2x2 = 8
            tc.tile_pool(name="pga", bufs=2, space="PSUM") as pga,
            tc.tile_pool(name="pph", bufs=1, space="PSUM") as pph,
            tc.tile_pool(name="pgt", bufs=2, space="PSUM") as pgt,
        ):
            # ---- constants ----
            wp1_sb = pc.tile([128, 2, H], F16)
            nc.sync.dma_start(out=wp1_sb, in_=wp1[:])
            w1_sb = pc.tile([128, 2, H], F16)
            nc.sync.dma_start(out=w1_sb, in_=w1[:])
            w2_sb = pc.tile([128, 2, H], F16)
            nc.sync.dma_start(out=w2_sb, in_=w2[:])
            qwh_sb = pc.tile([128, 2], F16)
            nc.sync.dma_start(out=qwh_sb, in_=qwh[:])
            bch_sb = pc.tile([128, 2], F32)
            nc.sync.dma_start(out=bch_sb, in_=bch[:])
            iota_at_sb = pc.tile([128, S], F16)
            nc.sync.dma_start(out=iota_at_sb, in_=iota_at[:])
            ident_sb = pc.tile([128, 128], F16)
            nc.sync.dma_start(out=ident_sb, in_=ident[:])
            iota_s_sb = pc.tile([S, 1], F32)
            nc.sync.dma_start(out=iota_s_sb, in_=iota_s[:])
            segc_sb = pc.tile([128, C, KT], F32)
            nc.sync.dma_start(out=segc_sb, in_=seg_col[:])
            rec_sb = pc.tile([S, C], F32)
            nc.sync.dma_start(out=rec_sb, in_=recip[:])

            # cross-iteration tile handles (software pipeline, 2-chunk skew)
            T_x, T_xT, T_pT, T_at, T_as, T_ga, T_smt, T_g1, T_ph, T_gt = (
                {}, {}, {}, {}, {}, {}, {}, {}, {}, {}
            )
            T_segb = {}
            T_hs2 = {}

            xtf = xt[:].rearrange("c t h -> (c t) h")
            pftf = pft[:].rearrange("c t h -> (c t) h")

            def emit_loads(c):
                # loads are pair-batched: one call covers chunks c and c+1
                np_ = min(2, C - c)
                nt = np_ * TC
                xT0 = pwk.tile([128, 2 * TC], F16, tag="xT0")
                nc.sync.dma_start_transpose(
                    out=xT0[:, :nt], in_=xtf[c * TC : c * TC + nt, 0:128]
                )
                xT1 = pwk.tile([128, 2 * TC], F16, tag="xT1")
                nc.sync.dma_start_transpose(
                    out=xT1[:, :nt], in_=xtf[c * TC : c * TC + nt, 128:256]
                )
                pT0 = pwk.tile([128, 2 * TC], F16, tag="pT0")
                nc.sync.dma_start_transpose(
                    out=pT0[:, :nt], in_=pftf[c * TC : c * TC + nt, 0:128]
                )
                pT1 = pwk.tile([128, 2 * TC], F16, tag="pT1")
                nc.sync.dma_start_transpose(
                    out=pT1[:, :nt], in_=pftf[c * TC : c * TC + nt, 128:256]
                )
                x = pwk.tile([128, 2 * KT, H], F16, tag="x")
                nc.sync.dma_start(
                    out=x[:, : np_ * KT, :],
                    in_=xtf[c * TC : c * TC + nt].rearrange(
                        "(k p) h -> p k h", p=128
                    ),
                )
                for j in range(np_):
                    T_x[c + j] = x[:, j * KT : (j + 1) * KT, :]
                    T_xT[c + j] = (
                        xT0[:, j * TC : (j + 1) * TC],
                        xT1[:, j * TC : (j + 1) * TC],
                    )
                    T_pT[c + j] = (
                        pT0[:, j * TC : (j + 1) * TC],
                        pT1[:, j * TC : (j + 1) * TC],
                    )
                if c % GRP == 0:
                    ng = min(GRP, C - c)
                    seg_src = seg_row[c]
                    segb = pwk.tile([S, GRP * TC], F16, tag="segb")
                    nc.sync.dma_start(
                        out=segb[:, : ng * TC],
                        in_=bass.AP(tensor=seg_src.tensor, offset=seg_src.offset,
                                    ap=[[0, S], [1, ng * TC]]),
                    )
                    T_segb[c // GRP] = segb

            emit_loads(0)
            if C > 2:
                emit_loads(2)
            for it in range(C + 2):
                c0 = it      # masks + ph + ss (+ tanh)
                c1 = it - 1  # g1 + gate (+ sigmoid)
                c2 = it - 2  # alpha + h_s
                if c0 % 2 == 0 and c0 + 4 < C:
                    emit_loads(c0 + 4)

                # ---- masks for c0 (DVE, feeds this iteration's ph/ss) ----
                if c0 < C:
                    gi = c0 % GRP
                    segb = T_segb[c0 // GRP]
                    a_s = pwk.tile([S, TC], F16, tag="a_s")
                    nc.vector.tensor_single_scalar(
                        out=a_s, in_=segb[:, gi * TC : (gi + 1) * TC],
                        scalar=iota_s_sb, op=eq,
                    )
                    a_t = pwk.tile([128, KT, S], F16, tag="a_t")
                    for k in range(KT):
                        nc.vector.tensor_single_scalar(
                            out=a_t[:, k, :], in_=iota_at_sb,
                            scalar=segc_sb[:, c0, k : k + 1], op=eq,
                        )
                    T_as[c0] = a_s
                    T_at[c0] = a_t

                # ---- alpha(c2): first PE work, deps one iteration old ----
                if c2 >= 0:
                    gt = T_gt.pop(c2)
                    gb = pgt.tile([128, 2 * TC], F32, tag="gate")
                    alp = gb[:, H : H + KT]
                    hsp = gb[0:S, 0:H]
                    for kt in range(KT):
                        for h in range(2):
                            nc.tensor.matmul(
                                alp[:, kt : kt + 1],
                                gt[:, h * TC + kt * 128 : h * TC + (kt + 1) * 128],
                                qwh_sb[:, h : h + 1],
                                start=(h == 0), stop=(h == 1),
                            )

                # ---- g1(c1) = (mean @ W1) * recip ----
                if 0 <= c1 < C:
                    smt = T_smt.pop(c1)
                    ga1 = T_ga[c1]
                    g1p = ga1[0:S, 2 * S : 2 * S + H]
                    for k in range(2):
                        nc.tensor.matmul(
                            g1p, smt[:, k * S : (k + 1) * S], w1_sb[:, k, :],
                            start=(k == 0), stop=(k == 1),
                        )
                    g1 = pwk.tile([S, H], F16, tag="g1")
                    nc.vector.tensor_single_scalar(
                        out=g1, in_=g1p, scalar=rec_sb[:, c1 : c1 + 1], op=mult
                    )
                    T_g1[c1] = g1
                    del T_ga[c1]

                # ---- ph(c0) = tanh(Wp1 @ x + pos_feat) ----
                if c0 < C:
                    php = pph.tile([128, 2 * TC], F32, tag="ph")
                    xTs = T_xT.pop(c0)
                    pTs = T_pT.pop(c0)
                    for h in range(2):
                        dst = php[:, h * TC : (h + 1) * TC]
                        lo, hi = h * 128, (h + 1) * 128
                        nc.tensor.matmul(dst, wp1_sb[:, 0, lo:hi], xTs[0],
                                         start=True, stop=False)
                        nc.tensor.matmul(dst, wp1_sb[:, 1, lo:hi], xTs[1],
                                         start=False, stop=False)
                        nc.tensor.matmul(dst, ident_sb, pTs[h],
                                         start=False, stop=True)
                    ph = pwk.tile([128, 2 * TC], F16, tag="ph_sb")
                    for h in range(2):
                        nc.scalar.activation(
                            out=ph[:, h * TC : (h + 1) * TC],
                            in_=php[:, h * TC : (h + 1) * TC],
                            func=Tanh,
                        )
                    T_ph[c0] = ph

                # ---- gate(c1) = tanh(0.5*(V + W2 @ ph) + bc/2) ----
                if 0 <= c1 < C:
                    ph1 = T_ph.pop(c1)
                    a_s1 = T_as.pop(c1)
                    g11 = T_g1.pop(c1)
                    gp = pgt.tile([128, 2 * TC], F32, tag="gate")
                    for h in range(2):
                        dst = gp[:, h * TC : (h + 1) * TC]
                        lo, hi = h * 128, (h + 1) * 128
                        nc.tensor.matmul(dst, g11[:, lo:hi], a_s1,
                                         start=True, stop=False)
                        nc.tensor.matmul(dst, w2_sb[:, 0, lo:hi], ph1[:, 0:TC],
                                         start=False, stop=False)
                        nc.tensor.matmul(dst, w2_sb[:, 1, lo:hi], ph1[:, TC:],
                                         start=False, stop=True)
                    gt1 = pwk.tile([128, 2 * TC], F16, tag="gt")
                    for h in range(2):
                        nc.scalar.activation(
                            out=gt1[:, h * TC : (h + 1) * TC],
                            in_=gp[:, h * TC : (h + 1) * TC],
                            func=Tanh, scale=0.5, bias=bch_sb[:, h : h + 1],
                        )
                    T_gt[c1] = gt1

                # ---- ss(c0): transposed session sums ----
                if c0 < C:
                    x0 = T_x[c0]
                    a_t0 = T_at[c0]
                    ga = pga.tile([128, 2 * S + H], F32, tag="ga")
                    ss = ga[:, 0 : 2 * S]
                    for h in range(2):
                        for k in range(KT):
                            nc.tensor.matmul(
                                ss[:, h * S : (h + 1) * S],
                                x0[:, k, h * 128 : (h + 1) * 128],
                                a_t0[:, k, :],
                                start=(k == 0),
                                stop=(k == KT - 1),
                            )
                    smt = pwk.tile([128, 2 * S], F16, tag="smt")
                    nc.vector.tensor_copy(out=smt, in_=ss)
                    T_ga[c0] = ga
                    T_smt[c0] = smt

                # ---- finish alpha(c2), h_s(c2) ----
                if c2 >= 0:
                    x2 = T_x.pop(c2)
                    a_t2 = T_at.pop(c2)
                    alpha = pwk.tile([128, KT], F32, tag="alpha")
                    nc.vector.tensor_scalar(
                        out=alpha, in0=alp, scalar1=0.5, scalar2=qbp,
                        op0=mult, op1=add,
                    )
                    aat = pwk.tile([128, KT, S], F16, tag="aat")
                    for k in range(KT):
                        nc.vector.tensor_single_scalar(
                            out=aat[:, k, :], in_=a_t2[:, k, :],
                            scalar=alpha[:, k : k + 1], op=mult,
                        )
                    for k in range(KT):
                        nc.tensor.matmul(
                            hsp, aat[:, k, :], x2[:, k, :],
                            start=(k == 0), stop=(k == KT - 1),
                        )
                    if c2 % 2 == 0:
                        hs2_new = pwk.tile([S, 2, H], F32, tag="hs2", name="hs2")
                        T_hs2[c2 // 2] = hs2_new
                    hs2 = T_hs2[c2 // 2]
                    nc.vector.tensor_copy(out=hs2[:, c2 % 2, :], in_=hsp)
                    if c2 % 2 == 1 or c2 == C - 1:
                        np_ = c2 % 2 + 1
                        lo_c = c2 - np_ + 1
                        # store via the idle GPSIMD SWDGE path: keeps both the
                        # SP and ACT HWDGE queues free for loads/activations
                        nc.gpsimd.dma_start(
                            out=hs[lo_c : c2 + 1].rearrange("p s h -> s p h"),
                            in_=hs2[:, :np_, :],
                        )
                        del T_hs2[c2 // 2]

    _split_multi_waits(nc)
    return nc


_CACHE = {}


def kernel(hidden, pos_table, W_pos, b_pos, W1, b1, W2, b2, qw, qb,
           seq_len, reverse_pos):
    hidden = np.asarray(hidden, np.float32)
    seq_len_np = np.asarray(seq_len)
    lens, starts, core_chunks, C = _plan(seq_len_np)
    weights, qbp, pw16 = _pack_weights(
        pos_table, W_pos, b_pos, W1, b1, W2, b2, qw, qb
    )
    xt16, pft16, seg_row, seg_col, recip, unpack_idx = _pack_inputs(
        hidden, reverse_pos, pw16, lens, starts, core_chunks, C
    )

    key = (C, qbp)
    if key not in _CACHE:
        _CACHE[key] = _build_bass(C, qbp)
    nc = _CACHE[key]

    in_maps = []
    for core in range(N_CORES):
        m = dict(
            xt=xt16[core], pft=pft16[core], seg_row=seg_row[core],
            seg_col=seg_col[core], recip=recip[core],
        )
        m.update(weights)
        in_maps.append(m)

    import time as _time

    t0 = _time.perf_counter()
    res = run_bass_kernel_spmd(nc, in_maps, core_ids=list(range(N_CORES)))
    kernel._last_run_s = _time.perf_counter() - t0
    hs_all = np.stack([res.results[i]["hs"] for i in range(N_CORES)])

    out_core, out_chunk, out_local = unpack_idx
    return np.ascontiguousarray(hs_all[out_core, out_chunk, out_local])



# revision 3
# speedup vs baseline: 2.2921x; 2.2921x over previous
"""Trainium2 Bass kernel for the CNNFusing ragged-session attention pooling module.

Computes, per session s over its token set:
    v_mean   = mean(hidden[s])                                  [H]
    ph[t]    = tanh(hidden[t] @ Wp1 + (pos_table @ Wp2 + b_pos)[rp[t]])
    gate[t]  = sigmoid(v_mean @ W1 + b1 + ph[t] @ W2 + b2)
    alpha[t] = gate[t] @ qw + qb
    h_s      = sum_t alpha[t] * hidden[t]                       [B, H]

Data-parallel over sessions on 8 cores.  Sessions are FFD-bin-packed into
512-token chunks (<=32 sessions per chunk).  Ragged ops become one-hot
matmuls.  The two big H x H GEMM chains per token (ph pre-act and the gate
ph-term) run as fp8e4m3 DoubleRow matmuls (2 contraction rows/partition);
session sums / weighted sums stay fp16.  h_s is accumulated transposed
([h, s] layout) so each accumulation step costs S output rows instead of H.
The sigmoid is folded into tanh and both per-token biases ride the one-hot
mean-term matmul, so each stage needs a single full-width tanh activation.
"""

import numpy as np
import ml_dtypes

import concourse.bass as bass
import concourse.mybir as mybir
import concourse.tile as tile
from concourse.bass_utils import run_bass_kernel_spmd

H = 256
TC = 512      # tokens per chunk
S = 32        # max sessions per chunk
KT = TC // 128
G = 4         # chunks per batched load/store DMA
SG = 8        # chunks per seg-row broadcast DMA
N_CORES = 8

F8 = mybir.dt.float8e4
F16 = mybir.dt.float16
F32 = mybir.dt.float32
NP_F8 = ml_dtypes.float8_e4m3fn


# --------------------------------------------------------------------------
# The walrus build here accepts only ONE sync-wait command per instruction,
# while Tile may attach several.  Hoist all but the last wait of such
# instructions onto standalone event-semaphore waits inserted just before
# them on the same engine (sequencer executes in order, semantics kept).
_waitsplit_uid = [0]


def _split_multi_waits(nc):
    for fn in nc.m.functions:
        for bb in fn.blocks:
            insts = bb.instructions
            i = 0
            while i < len(insts):
                inst = insts[i]
                si = getattr(inst, "sync_info", None)
                waits = list(si.on_wait) if si is not None and si.on_wait else []
                if len(waits) > 1:
                    si.on_wait = waits[-1:]
                    for w in waits[:-1]:
                        ev = mybir.InstEventSemaphore(
                            name=f"I-waitsplit-{_waitsplit_uid[0]}", ins=[], outs=[]
                        )
                        _waitsplit_uid[0] += 1
                        ev.engine = inst.engine
                        ev.sync_info = mybir.SyncInfo(on_wait=[w], on_update=[])
                        insts.insert(i, ev)
                        i += 1
                i += 1
# --------------------------------------------------------------------------


def _plan(seq_len):
    """Best-fit-decreasing bin packing of all sessions into (token<=TC,
    sessions<=S) chunks, then deal chunks round-robin to cores."""
    lens = np.asarray(seq_len, dtype=np.int64)
    B = len(lens)
    order = np.argsort(-lens, kind="stable")
    bins = []          # list of [tok_used, [session ids]]
    # rem_sorted: sorted list of (remaining_tokens, bin_idx) for best-fit
    import bisect
    rem = []           # sorted (remaining, bin_idx)
    for sid in order:
        L = int(lens[sid])
        # best fit: smallest remaining >= L
        pos = bisect.bisect_left(rem, (L, -1))
        placed = False
        while pos < len(rem):
            r, bi = rem[pos]
            if len(bins[bi][1]) < S:
                rem.pop(pos)
                bins[bi][0] += L
                bins[bi][1].append(sid)
                nr = TC - bins[bi][0]
                if nr > 0:
                    bisect.insort(rem, (nr, bi))
                placed = True
                break
            pos += 1
        if not placed:
            bi = len(bins)
            bins.append([L, [sid]])
            bisect.insort(rem, (TC - L, bi))
    nb = len(bins)
    C = -(-nb // N_CORES)
    core_chunks = [[] for _ in range(N_CORES)]
    for i, b in enumerate(bins):
        core_chunks[i % N_CORES].append(b[1])
    return lens, core_chunks, C


def _pack_inputs(hidden, reverse_pos, pw8, lens, core_chunks, C):
    """Build all per-core DRAM input arrays."""
    B = len(lens)
    starts = np.concatenate([[0], np.cumsum(lens)[:-1]])
    hidden16 = np.asarray(hidden, np.float32).astype(np.float16)
    rp = np.asarray(reverse_pos)

    # token map [N, C, TC] -> global token index (or -1)
    tokmap = np.full((N_CORES, C, TC), -1, np.int64)
    seg_row = np.full((N_CORES, C, TC), -1.0, np.float16)
    recip = np.zeros((N_CORES, S, C), np.float32)
    out_core = np.zeros(B, np.int32)
    out_chunk = np.zeros(B, np.int32)
    out_local = np.zeros(B, np.int32)

    for core in range(N_CORES):
        for ci, sess in enumerate(core_chunks[core]):
            t = 0
            for si, sid in enumerate(sess):
                L = int(lens[sid])
                tokmap[core, ci, t : t + L] = np.arange(starts[sid], starts[sid] + L)
                seg_row[core, ci, t : t + L] = si
                recip[core, si, ci] = 1.0 / L
                out_core[sid] = core
                out_chunk[sid] = ci
                out_local[sid] = si
                t += L

    valid = tokmap >= 0
    idx = np.where(valid, tokmap, 0)

    # gathered hidden [N, C, TC, H] fp16 (zero padded)
    xt = hidden16[idx]
    xt[~valid] = 0
    # row tiles [N, C, 128, KT, H] f16
    x16 = np.ascontiguousarray(
        xt.reshape(N_CORES, C, KT, 128, H).transpose(0, 1, 3, 2, 4)
    )
    # transposed fp8 [N, C, 128, 2, TC]
    xt8 = np.ascontiguousarray(
        xt.astype(NP_F8).transpose(0, 1, 3, 2).reshape(N_CORES, C, 2, 128, TC)
        .transpose(0, 1, 3, 2, 4)
    )
    del xt

    # pos features (already fp8-quantized table), gathered transposed
    rpg = np.where(valid, rp[idx], 0)
    pft = pw8[rpg]                                  # [N, C, TC, H] fp8
    pft[~valid] = 0
    pf8 = np.ascontiguousarray(
        pft.transpose(0, 1, 3, 2).reshape(N_CORES, C, 2, 128, TC)
        .transpose(0, 1, 3, 2, 4)
    )
    del pft

    seg_col = np.ascontiguousarray(
        seg_row.reshape(N_CORES, C, KT, 128).transpose(0, 3, 1, 2)
    ).astype(np.float32)

    return x16, xt8, pf8, seg_row, seg_col, recip, (out_core, out_chunk, out_local)


def _pack_weights(pos_table, W_pos, b_pos, W1, b1, W2, b2, qw, qb):
    Wp = np.asarray(W_pos, np.float32)
    pwf = np.asarray(pos_table, np.float32) @ Wp[H:] + np.asarray(b_pos, np.float32)
    pw8 = np.zeros((H, H), NP_F8)
    pw8[: pwf.shape[0]] = pwf.astype(NP_F8)

    def pack_dr(M):  # [256, 256] -> [128, 2, 256] fp8, row c = 128*i + p
        return np.ascontiguousarray(
            np.asarray(M, np.float32).reshape(2, 128, H).transpose(1, 0, 2)
        ).astype(NP_F8)

    wp18 = pack_dr(Wp[:H])
    w28 = pack_dr(np.asarray(W2, np.float32))
    w18 = pack_dr(np.asarray(W1, np.float32))

    ident8 = np.zeros((128, 2, H), NP_F8)
    for m in range(2):
        ident8[:, m, m * 128 : (m + 1) * 128] = np.eye(128, dtype=NP_F8)

    qwf = np.asarray(qw, np.float32).reshape(H)
    qwh = np.ascontiguousarray(qwf.reshape(2, 128).T).astype(np.float16)
    qbp = float(np.asarray(qb, np.float32).reshape(()) + qwf.sum() / 2.0)
    # full (unscaled) bias b1+b2 rides the mean-term; ACT applies tanh(z/2)
    bcf = np.asarray(b1, np.float32) + np.asarray(b2, np.float32)
    bchrow = np.broadcast_to(bcf, (S, H)).copy().astype(np.float32)

    iota_at = np.broadcast_to(np.arange(S, dtype=np.float16), (128, S)).copy()
    iota_s = np.arange(S, dtype=np.float32).reshape(S, 1)
    return dict(
        wp18=wp18, w28=w28, w18=w18, ident8=ident8, qwh=qwh, bchrow=bchrow,
        iota_at=iota_at, iota_s=iota_s,
    ), qbp, pw8


def _build_bass(C, qbp):
    nc = bass.Bass("TRN2", target_bir_lowering=False, debug=False,
                   num_devices=N_CORES)

    x16 = nc.dram_tensor("x16", [C, 128, KT, H], F16, kind="ExternalInput")
    xt8 = nc.dram_tensor("xt8", [C, 128, 2, TC], F8, kind="ExternalInput")
    pf8 = nc.dram_tensor("pf8", [C, 128, 2, TC], F8, kind="ExternalInput")
    seg_row = nc.dram_tensor("seg_row", [C, TC], F16, kind="ExternalInput")
    seg_col = nc.dram_tensor("seg_col", [128, C, KT], F32, kind="ExternalInput")
    recip = nc.dram_tensor("recip", [S, C], F32, kind="ExternalInput")
    wp18 = nc.dram_tensor("wp18", [128, 2, H], F8, kind="ExternalInput")
    w28 = nc.dram_tensor("w28", [128, 2, H], F8, kind="ExternalInput")
    w18 = nc.dram_tensor("w18", [128, 2, H], F8, kind="ExternalInput")
    ident8 = nc.dram_tensor("ident8", [128, 2, H], F8, kind="ExternalInput")
    qwh = nc.dram_tensor("qwh", [128, 2], F16, kind="ExternalInput")
    bchrow = nc.dram_tensor("bchrow", [S, H], F32, kind="ExternalInput")
    iota_at = nc.dram_tensor("iota_at", [128, S], F16, kind="ExternalInput")
    iota_s = nc.dram_tensor("iota_s", [S, 1], F32, kind="ExternalInput")
    hst = nc.dram_tensor("hst", [128, C, 2 * S], F32, kind="ExternalOutput")

    eq = mybir.AluOpType.is_equal
    mult = mybir.AluOpType.mult
    add = mybir.AluOpType.add
    Tanh = mybir.ActivationFunctionType.Tanh
    DR = mybir.MatmulPerfMode.DoubleRow

    NG = -(-C // G)    # number of load groups

    with tile.TileContext(nc) as tc:
        with (
            tc.tile_pool(name="consts", bufs=1) as pc,
            tc.tile_pool(name="loads", bufs=3) as pl,
            tc.tile_pool(name="segp", bufs=2) as psg,
            tc.tile_pool(name="work", bufs=5) as pwk,
            # PSUM: ph 1x2 banks + ga 2x1 + gate 2x2 = 8 banks
            tc.tile_pool(name="pph", bufs=1, space="PSUM") as pph,
            tc.tile_pool(name="pga", bufs=2, space="PSUM") as pga,
            tc.tile_pool(name="pgt", bufs=2, space="PSUM") as pgt,
        ):
            # ---- constants ----
            wp18_sb = pc.tile([128, 2, H], F8)
            nc.sync.dma_start(out=wp18_sb, in_=wp18[:])
            w28_sb = pc.tile([128, 2, H], F8)
            nc.sync.dma_start(out=w28_sb, in_=w28[:])
            w18_sb = pc.tile([128, 2, H], F8)
            nc.sync.dma_start(out=w18_sb, in_=w18[:])
            id8_sb = pc.tile([128, 2, H], F8)
            nc.sync.dma_start(out=id8_sb, in_=ident8[:])
            qwh_sb = pc.tile([128, 2], F16)
            nc.sync.dma_start(out=qwh_sb, in_=qwh[:])
            bch_sb = pc.tile([S, H], F32)
            nc.sync.dma_start(out=bch_sb, in_=bchrow[:])
            iota_at_sb = pc.tile([128, S], F16)
            nc.sync.dma_start(out=iota_at_sb, in_=iota_at[:])
            iota_s_sb = pc.tile([S, 1], F32)
            nc.sync.dma_start(out=iota_s_sb, in_=iota_s[:])
            segc_sb = pc.tile([128, C, KT], F32)
            nc.sync.dma_start(out=segc_sb, in_=seg_col[:])
            rec_sb = pc.tile([S, C], F32)
            nc.sync.dma_start(out=rec_sb, in_=recip[:])

            T_x16, T_xt8, T_pf8 = {}, {}, {}
            T_segb = {}
            T_as, T_at, T_ph8, T_g1, T_smt, T_gt, T_al, T_aat = (
                {}, {}, {}, {}, {}, {}, {}, {}
            )
            T_hsg = {}

            def emit_loads(g):
                c = g * G
                ng = min(G, C - c)
                x16_t = pl.tile([128, G, KT, H], F16, tag="x16")
                nc.sync.dma_start(
                    out=x16_t[:, :ng], in_=x16[c : c + ng].rearrange("c p k h -> p c k h")
                )
                xt8_t = pl.tile([128, G, 2, TC], F8, tag="xt8")
                nc.sync.dma_start(
                    out=xt8_t[:, :ng], in_=xt8[c : c + ng].rearrange("c p i t -> p c i t")
                )
                pf8_t = pl.tile([128, G, 2, TC], F8, tag="pf8")
                nc.sync.dma_start(
                    out=pf8_t[:, :ng], in_=pf8[c : c + ng].rearrange("c p i t -> p c i t")
                )
                for j in range(ng):
                    T_x16[c + j] = x16_t[:, j]
                    T_xt8[c + j] = xt8_t[:, j]
                    T_pf8[c + j] = pf8_t[:, j]

            def emit_seg(sg):
                c = sg * SG
                n = min(SG, C - c)
                src = seg_row[c]
                segb = psg.tile([S, SG * TC], F16, tag="segb")
                nc.sync.dma_start(
                    out=segb[:, : n * TC],
                    in_=bass.AP(tensor=src.tensor, offset=src.offset,
                                ap=[[0, S], [1, n * TC]]),
                )
                T_segb[sg] = segb

            emit_loads(0)
            if NG > 1:
                emit_loads(1)
            emit_seg(0)

            for it in range(C + 3):
                c0, c1, c2, c3 = it, it - 1, it - 2, it - 3

                # prefetch
                if c0 % G == 0 and c0 // G + 2 < NG:
                    emit_loads(c0 // G + 2)
                if c0 % SG == 0 and c0 // SG + 1 <= (C - 1) // SG:
                    emit_seg(c0 // SG + 1)

                # ---- alpha(c3): matmuls into gate tile, then DVE scale+aat
                if 0 <= c3 < C:
                    gt3 = T_gt.pop(c3)
                    gb3 = T_al.pop(c3)       # gate psum tile of c3
                    alp = gb3[:, 0:KT]
                    for k in range(KT):
                        for h in range(2):
                            nc.tensor.matmul(
                                alp[:, k : k + 1],
                                gt3[:, h * TC + k * 128 : h * TC + (k + 1) * 128],
                                qwh_sb[:, h : h + 1],
                                start=(h == 0), stop=(h == 1),
                            )
                    alpha = pwk.tile([128, KT], F32, tag="alpha")
                    nc.vector.tensor_scalar(
                        out=alpha, in0=alp, scalar1=0.5, scalar2=qbp,
                        op0=mult, op1=add,
                    )
                    a_t3 = T_at.pop(c3)
                    aat = pwk.tile([128, KT, S], F16, tag="aat")
                    nc.vector.tensor_tensor(
                        out=aat,
                        in0=a_t3,
                        in1=bass.AP(tensor=alpha.tensor, offset=alpha.offset,
                                    ap=[list(alpha.ap[0]), [1, KT], [0, S]]),
                        op=mult,
                    )
                    T_aat[c3] = (gb3, aat)

                # ---- gate(c2) matmuls + tanh
                if 0 <= c2 < C:
                    g1_2 = T_g1.pop(c2)
                    a_s2 = T_as.pop(c2)
                    ph8_2 = T_ph8.pop(c2)
                    gp = pgt.tile([128, 2 * TC], F32, tag="gate")
                    for h in range(2):
                        dst = gp[:, h * TC : (h + 1) * TC]
                        nc.tensor.matmul(
                            dst, g1_2[:, h * 128 : (h + 1) * 128], a_s2,
                            start=True, stop=False,
                        )
                        nc.tensor.matmul(
                            dst,
                            w28_sb[:, :, h * 128 : (h + 1) * 128],
                            ph8_2.rearrange("p (i t) -> p i t", i=2),
                            start=False, stop=True, perf_mode=DR,
                        )
                    gt2 = pwk.tile([128, 2 * TC], F16, tag="gt")
                    nc.scalar.activation(out=gt2, in_=gp, func=Tanh, scale=0.5)
                    T_gt[c2] = gt2
                    T_al[c2] = gp

                # ---- g1(c1): DR matmul + scale/bias
                if 0 <= c1 < C:
                    smt1, ga1 = T_smt.pop(c1)
                    g1p = ga1[0:S, 2 * S : 2 * S + H]
                    nc.tensor.matmul(
                        g1p,
                        smt1.rearrange("p (i s) -> p i s", i=2),
                        w18_sb[:],
                        start=True, stop=True, perf_mode=DR,
                    )
                    g1 = pwk.tile([S, H], F16, tag="g1")
                    nc.vector.scalar_tensor_tensor(
                        out=g1, in0=g1p, scalar=rec_sb[:, c1 : c1 + 1],
                        in1=bch_sb, op0=mult, op1=add,
                    )
                    T_g1[c1] = g1

                # ---- masks(c0)
                if c0 < C:
                    sg, si = c0 // SG, c0 % SG
                    segb = T_segb[sg]
                    a_s = pwk.tile([S, TC], F16, tag="a_s")
                    nc.vector.tensor_single_scalar(
                        out=a_s, in_=segb[:, si * TC : (si + 1) * TC],
                        scalar=iota_s_sb, op=eq,
                    )
                    a_t = pwk.tile([128, KT, S], F16, tag="a_t")
                    nc.vector.tensor_tensor(
                        out=a_t,
                        in0=bass.AP(tensor=iota_at_sb.tensor, offset=iota_at_sb.offset,
                                    ap=[list(iota_at_sb.ap[0]), [0, KT], [1, S]]),
                        in1=bass.AP(tensor=segc_sb.tensor,
                                    offset=segc_sb.offset + c0 * KT,
                                    ap=[list(segc_sb.ap[0]), [1, KT], [0, S]]),
                        op=eq,
                    )
                    T_as[c0] = a_s
                    T_at[c0] = a_t

                # ---- ph(c0): DR matmuls + tanh -> fp8
                if c0 < C:
                    xt8_0 = T_xt8.pop(c0)
                    pf8_0 = T_pf8.pop(c0)
                    php = pph.tile([128, 2 * TC], F32, tag="ph")
                    for h in range(2):
                        dst = php[:, h * TC : (h + 1) * TC]
                        nc.tensor.matmul(
                            dst, wp18_sb[:, :, h * 128 : (h + 1) * 128], xt8_0,
                            start=True, stop=False, perf_mode=DR,
                        )
                        nc.tensor.matmul(
                            dst, id8_sb[:, :, h * 128 : (h + 1) * 128], pf8_0,
                            start=False, stop=True, perf_mode=DR,
                        )
                    ph8 = pwk.tile([128, 2 * TC], F8, tag="ph8")
                    nc.scalar.activation(out=ph8, in_=php, func=Tanh)
                    T_ph8[c0] = ph8

                # ---- ss(c0): transposed session sums + fp8 copy
                if c0 < C:
                    x16_0 = T_x16[c0]
                    a_t0 = T_at[c0]
                    ga = pga.tile([128, 2 * S + H], F32, tag="ga")
                    ss = ga[:, 0 : 2 * S]
                    for h in range(2):
                        for k in range(KT):
                            nc.tensor.matmul(
                                ss[:, h * S : (h + 1) * S],
                                x16_0[:, k, h * 128 : (h + 1) * 128],
                                a_t0[:, k, :],
                                start=(k == 0), stop=(k == KT - 1),
                            )
                    smt = pwk.tile([128, 2 * S], F8, tag="smt")
                    nc.vector.tensor_copy(out=smt, in_=ss)
                    T_smt[c0] = (smt, ga)

                # ---- h_s(c3): transposed weighted sums, copy, store
                if 0 <= c3 < C:
                    gb3, aat3 = T_aat.pop(c3)
                    x16_3 = T_x16.pop(c3)
                    hsp = gb3[:, TC : TC + 2 * S]
                    for h in range(2):
                        for k in range(KT):
                            nc.tensor.matmul(
                                hsp[:, h * S : (h + 1) * S],
                                x16_3[:, k, h * 128 : (h + 1) * 128],
                                aat3[:, k, :],
                                start=(k == 0), stop=(k == KT - 1),
                            )
                    if c3 % G == 0:
                        T_hsg[c3 // G] = pwk.tile([128, G, 2 * S], F32, tag="hsg",
                                                  name="hsg")
                    hsg = T_hsg[c3 // G]
                    nc.vector.tensor_copy(out=hsg[:, c3 % G], in_=hsp)
                    if c3 % G == G - 1 or c3 == C - 1:
                        n = c3 % G + 1
                        lo = c3 - n + 1
                        nc.gpsimd.dma_start(
                            out=hst[:, lo : c3 + 1, :], in_=hsg[:, :n],
                        )
                        del T_hsg[c3 // G]

    _split_multi_waits(nc)
    return nc


_CACHE = {}


def kernel(hidden, pos_table, W_pos, b_pos, W1, b1, W2, b2, qw, qb,
           seq_len, reverse_pos):
    seq_len_np = np.asarray(seq_len)
    lens, core_chunks, C = _plan(seq_len_np)
    weights, qbp, pw8 = _pack_weights(
        pos_table, W_pos, b_pos, W1, b1, W2, b2, qw, qb
    )
    x16, xt8, pf8, seg_row, seg_col, recip, unpack_idx = _pack_inputs(
        hidden, reverse_pos, pw8, lens, core_chunks, C
    )

    key = (C, qbp)
    if key not in _CACHE:
        _CACHE[key] = _build_bass(C, qbp)
    nc = _CACHE[key]

    in_maps = []
    for core in range(N_CORES):
        m = dict(
            x16=x16[core], xt8=xt8[core], pf8=pf8[core],
            seg_row=seg_row[core], seg_col=seg_col[core], recip=recip[core],
        )
        m.update(weights)
        in_maps.append(m)

    import time as _time

    t0 = _time.perf_counter()
    res = run_bass_kernel_spmd(nc, in_maps, core_ids=list(range(N_CORES)))
    kernel._last_run_s = _time.perf_counter() - t0
    # hst: [N, 128, C, 2S] f32 -> h_s[sess, h] with h = 128*half + p
    hs_all = np.stack([res.results[i]["hst"] for i in range(N_CORES)])
    hs_all = hs_all.reshape(N_CORES, 128, C, 2, S)

    out_core, out_chunk, out_local = unpack_idx
    # [sess, half, p] -> [sess, 128*half + p]
    out = hs_all[out_core, :, out_chunk, :, out_local]      # [B, 128, 2]
    out = out.transpose(0, 2, 1).reshape(len(out_core), H)
    return np.ascontiguousarray(out)


# revision 16
# speedup vs baseline: 2.3494x; 1.0250x over previous
"""Trainium2 Bass kernel for the CNNFusing ragged-session attention pooling module.

Computes, per session s over its token set:
    v_mean   = mean(hidden[s])                                  [H]
    ph[t]    = tanh(hidden[t] @ Wp1 + (pos_table @ Wp2 + b_pos)[rp[t]])
    gate[t]  = sigmoid(v_mean @ W1 + b1 + ph[t] @ W2 + b2)
    alpha[t] = gate[t] @ qw + qb
    h_s      = sum_t alpha[t] * hidden[t]                       [B, H]

Data-parallel over sessions on 8 cores.  Sessions are FFD-bin-packed into
512-token chunks (<=32 sessions per chunk).  Ragged ops become one-hot
matmuls.  The two big H x H GEMM chains per token (ph pre-act and the gate
ph-term) run as fp8e4m3 DoubleRow matmuls (2 contraction rows/partition);
session sums / weighted sums stay fp16.  h_s is accumulated transposed
([h, s] layout) so each accumulation step costs S output rows instead of H.
The sigmoid is folded into tanh and both per-token biases ride the one-hot
mean-term matmul, so each stage needs a single full-width tanh activation.
"""

import numpy as np
import ml_dtypes

import concourse.bass as bass
import concourse.mybir as mybir
import concourse.tile as tile
from concourse.bass_utils import run_bass_kernel_spmd

H = 256
TC = 512      # tokens per chunk
S = 32        # max sessions per chunk
KT = TC // 128
G = 2         # chunks per batched load/store DMA
SG = 8        # chunks per seg-row broadcast DMA
N_CORES = 8

F8 = mybir.dt.float8e4
F16 = mybir.dt.float16
F32 = mybir.dt.float32
NP_F8 = ml_dtypes.float8_e4m3fn


# --------------------------------------------------------------------------
# The walrus build here accepts only ONE sync-wait command per instruction,
# while Tile may attach several.  Hoist all but the last wait of such
# instructions onto standalone event-semaphore waits inserted just before
# them on the same engine (sequencer executes in order, semantics kept).
_waitsplit_uid = [0]


def _split_multi_waits(nc):
    for fn in nc.m.functions:
        for bb in fn.blocks:
            insts = bb.instructions
            i = 0
            while i < len(insts):
                inst = insts[i]
                si = getattr(inst, "sync_info", None)
                waits = list(si.on_wait) if si is not None and si.on_wait else []
                if len(waits) > 1:
                    si.on_wait = waits[-1:]
                    for w in waits[:-1]:
                        ev = mybir.InstEventSemaphore(
                            name=f"I-waitsplit-{_waitsplit_uid[0]}", ins=[], outs=[]
                        )
                        _waitsplit_uid[0] += 1
                        ev.engine = inst.engine
                        ev.sync_info = mybir.SyncInfo(on_wait=[w], on_update=[])
                        insts.insert(i, ev)
                        i += 1
                i += 1
# --------------------------------------------------------------------------


def _plan(seq_len):
    """Best-fit-decreasing bin packing of all sessions into (token<=TC,
    sessions<=S) chunks, then deal chunks round-robin to cores."""
    lens = np.asarray(seq_len, dtype=np.int64)
    B = len(lens)
    order = np.argsort(-lens, kind="stable")
    bins = []          # list of [tok_used, [session ids]]
    # rem_sorted: sorted list of (remaining_tokens, bin_idx) for best-fit
    import bisect
    rem = []           # sorted (remaining, bin_idx)
    for sid in order:
        L = int(lens[sid])
        # best fit: smallest remaining >= L
        pos = bisect.bisect_left(rem, (L, -1))
        placed = False
        while pos < len(rem):
            r, bi = rem[pos]
            if len(bins[bi][1]) < S:
                rem.pop(pos)
                bins[bi][0] += L
                bins[bi][1].append(sid)
                nr = TC - bins[bi][0]
                if nr > 0:
                    bisect.insort(rem, (nr, bi))
                placed = True
                break
            pos += 1
        if not placed:
            bi = len(bins)
            bins.append([L, [sid]])
            bisect.insort(rem, (TC - L, bi))
    nb = len(bins)
    C = -(-nb // N_CORES)
    core_chunks = [[] for _ in range(N_CORES)]
    for i, b in enumerate(bins):
        core_chunks[i % N_CORES].append(b[1])
    return lens, core_chunks, C


def _pack_inputs(hidden, reverse_pos, pw8, lens, core_chunks, C):
    """Build all per-core DRAM input arrays."""
    B = len(lens)
    starts = np.concatenate([[0], np.cumsum(lens)[:-1]])
    hidden16 = np.asarray(hidden, np.float32).astype(np.float16)
    rp = np.asarray(reverse_pos)

    # token map [N, C, TC] -> global token index (or -1)
    tokmap = np.full((N_CORES, C, TC), -1, np.int64)
    seg_row = np.full((N_CORES, C, TC), -1.0, np.float16)
    recip = np.zeros((N_CORES, S, C), np.float32)
    out_core = np.zeros(B, np.int32)
    out_chunk = np.zeros(B, np.int32)
    out_local = np.zeros(B, np.int32)

    for core in range(N_CORES):
        for ci, sess in enumerate(core_chunks[core]):
            t = 0
            for si, sid in enumerate(sess):
                L = int(lens[sid])
                tokmap[core, ci, t : t + L] = np.arange(starts[sid], starts[sid] + L)
                seg_row[core, ci, t : t + L] = si
                recip[core, si, ci] = 1.0 / L
                out_core[sid] = core
                out_chunk[sid] = ci
                out_local[sid] = si
                t += L

    valid = tokmap >= 0
    idx = np.where(valid, tokmap, 0)

    # gathered hidden [N, C, TC, H] fp16 (zero padded)
    xt = hidden16[idx]
    xt[~valid] = 0
    # row tiles [N, C, 128, KT, H] f16
    x16 = np.ascontiguousarray(
        xt.reshape(N_CORES, C, KT, 128, H).transpose(0, 1, 3, 2, 4)
    )
    # transposed fp8 [N, C, 128, 2, TC]
    xt8 = np.ascontiguousarray(
        xt.astype(NP_F8).transpose(0, 1, 3, 2).reshape(N_CORES, C, 2, 128, TC)
        .transpose(0, 1, 3, 2, 4)
    )
    del xt

    # pos features (already fp8-quantized table), gathered transposed
    rpg = np.where(valid, rp[idx], 0)
    pft = pw8[rpg]                                  # [N, C, TC, H] fp8
    pft[~valid] = 0
    pf8 = np.ascontiguousarray(
        pft.transpose(0, 1, 3, 2).reshape(N_CORES, C, 2, 128, TC)
        .transpose(0, 1, 3, 2, 4)
    )
    del pft

    seg_col = np.ascontiguousarray(
        seg_row.reshape(N_CORES, C, KT, 128).transpose(0, 3, 1, 2)
    ).astype(np.float32)

    return x16, xt8, pf8, seg_row, seg_col, recip, (out_core, out_chunk, out_local)


def _pack_weights(pos_table, W_pos, b_pos, W1, b1, W2, b2, qw, qb):
    Wp = np.asarray(W_pos, np.float32)
    pwf = np.asarray(pos_table, np.float32) @ Wp[H:] + np.asarray(b_pos, np.float32)
    pw8 = np.zeros((H, H), NP_F8)
    pw8[: pwf.shape[0]] = pwf.astype(NP_F8)

    def pack_dr(M):  # [256, 256] -> [128, 2, 256] fp8, row c = 128*i + p
        return np.ascontiguousarray(
            np.asarray(M, np.float32).reshape(2, 128, H).transpose(1, 0, 2)
        ).astype(NP_F8)

    wp18 = pack_dr(Wp[:H])
    w28 = pack_dr(np.asarray(W2, np.float32))
    w18 = pack_dr(np.asarray(W1, np.float32))

    ident8 = np.zeros((128, 2, H), NP_F8)
    for m in range(2):
        ident8[:, m, m * 128 : (m + 1) * 128] = np.eye(128, dtype=NP_F8)

    qwf = np.asarray(qw, np.float32).reshape(H)
    qwh = np.ascontiguousarray(qwf.reshape(2, 128).T).astype(np.float16)
    qbp = float(np.asarray(qb, np.float32).reshape(()) + qwf.sum() / 2.0)
    # full (unscaled) bias b1+b2 rides the mean-term; ACT applies tanh(z/2)
    bcf = np.asarray(b1, np.float32) + np.asarray(b2, np.float32)
    bchrow = np.broadcast_to(bcf, (S, H)).copy().astype(np.float32)

    iota_at = np.broadcast_to(np.arange(S, dtype=np.float16), (128, S)).copy()
    iota_s = np.arange(S, dtype=np.float32).reshape(S, 1)
    return dict(
        wp18=wp18, w28=w28, w18=w18, ident8=ident8, qwh=qwh, bchrow=bchrow,
        iota_at=iota_at, iota_s=iota_s,
    ), qbp, pw8


def _build_bass(C, qbp):
    nc = bass.Bass("TRN2", target_bir_lowering=False, debug=False,
                   num_devices=N_CORES)

    x16 = nc.dram_tensor("x16", [C, 128, KT, H], F16, kind="ExternalInput")
    xt8 = nc.dram_tensor("xt8", [C, 128, 2, TC], F8, kind="ExternalInput")
    pf8 = nc.dram_tensor("pf8", [C, 128, 2, TC], F8, kind="ExternalInput")
    seg_row = nc.dram_tensor("seg_row", [C, TC], F16, kind="ExternalInput")
    seg_col = nc.dram_tensor("seg_col", [128, C, KT], F32, kind="ExternalInput")
    recip = nc.dram_tensor("recip", [S, C], F32, kind="ExternalInput")
    wp18 = nc.dram_tensor("wp18", [128, 2, H], F8, kind="ExternalInput")
    w28 = nc.dram_tensor("w28", [128, 2, H], F8, kind="ExternalInput")
    w18 = nc.dram_tensor("w18", [128, 2, H], F8, kind="ExternalInput")
    ident8 = nc.dram_tensor("ident8", [128, 2, H], F8, kind="ExternalInput")
    qwh = nc.dram_tensor("qwh", [128, 2], F16, kind="ExternalInput")
    bchrow = nc.dram_tensor("bchrow", [S, H], F32, kind="ExternalInput")
    iota_at = nc.dram_tensor("iota_at", [128, S], F16, kind="ExternalInput")
    iota_s = nc.dram_tensor("iota_s", [S, 1], F32, kind="ExternalInput")
    hst = nc.dram_tensor("hst", [128, C, 2 * S], F32, kind="ExternalOutput")

    eq = mybir.AluOpType.is_equal
    mult = mybir.AluOpType.mult
    add = mybir.AluOpType.add
    Tanh = mybir.ActivationFunctionType.Tanh
    DR = mybir.MatmulPerfMode.DoubleRow

    NG = -(-C // G)    # number of load groups

    with tile.TileContext(nc) as tc:
        with (
            tc.tile_pool(name="consts", bufs=1) as pc,
            tc.tile_pool(name="loads", bufs=4) as pl,
            tc.tile_pool(name="segp", bufs=2) as psg,
            tc.tile_pool(name="work", bufs=5) as pwk,
            # PSUM: ph 1x2 banks + ga 2x1 + gate 2x2 = 8 banks
            tc.tile_pool(name="pph", bufs=1, space="PSUM") as pph,
            tc.tile_pool(name="pga", bufs=2, space="PSUM") as pga,
            tc.tile_pool(name="pgt", bufs=2, space="PSUM") as pgt,
        ):
            # ---- constants ----
            wp18_sb = pc.tile([128, 2, H], F8)
            nc.sync.dma_start(out=wp18_sb, in_=wp18[:])
            w28_sb = pc.tile([128, 2, H], F8)
            nc.sync.dma_start(out=w28_sb, in_=w28[:])
            w18_sb = pc.tile([128, 2, H], F8)
            nc.sync.dma_start(out=w18_sb, in_=w18[:])
            id8_sb = pc.tile([128, 2, H], F8)
            nc.sync.dma_start(out=id8_sb, in_=ident8[:])
            qwh_sb = pc.tile([128, 2], F16)
            nc.sync.dma_start(out=qwh_sb, in_=qwh[:])
            bch_sb = pc.tile([S, H], F32)
            nc.sync.dma_start(out=bch_sb, in_=bchrow[:])
            iota_at_sb = pc.tile([128, S], F16)
            nc.sync.dma_start(out=iota_at_sb, in_=iota_at[:])
            iota_s_sb = pc.tile([S, 1], F32)
            nc.sync.dma_start(out=iota_s_sb, in_=iota_s[:])
            segc_sb = pc.tile([128, C, KT], F32)
            nc.sync.dma_start(out=segc_sb, in_=seg_col[:])
            rec_sb = pc.tile([S, C], F32)
            nc.sync.dma_start(out=rec_sb, in_=recip[:])

            T_x16, T_xt8, T_pf8 = {}, {}, {}
            T_segb = {}
            T_as, T_at, T_ph8, T_g1, T_smt, T_gt, T_al, T_aat = (
                {}, {}, {}, {}, {}, {}, {}, {}
            )
            T_hsg = {}

            def emit_loads(g):
                c = g * G
                ng = min(G, C - c)
                x16_t = pl.tile([128, G, KT, H], F16, tag="x16")
                nc.sync.dma_start(
                    out=x16_t[:, :ng], in_=x16[c : c + ng].rearrange("c p k h -> p c k h")
                )
                xt8_t = pl.tile([128, G, 2, TC], F8, tag="xt8")
                nc.sync.dma_start(
                    out=xt8_t[:, :ng], in_=xt8[c : c + ng].rearrange("c p i t -> p c i t")
                )
                pf8_t = pl.tile([128, G, 2, TC], F8, tag="pf8")
                nc.sync.dma_start(
                    out=pf8_t[:, :ng], in_=pf8[c : c + ng].rearrange("c p i t -> p c i t")
                )
                for j in range(ng):
                    T_x16[c + j] = x16_t[:, j]
                    T_xt8[c + j] = xt8_t[:, j]
                    T_pf8[c + j] = pf8_t[:, j]

            def emit_seg(sg):
                c = sg * SG
                n = min(SG, C - c)
                src = seg_row[c]
                segb = psg.tile([S, SG * TC], F16, tag="segb")
                nc.sync.dma_start(
                    out=segb[:, : n * TC],
                    in_=bass.AP(tensor=src.tensor, offset=src.offset,
                                ap=[[0, S], [1, n * TC]]),
                )
                T_segb[sg] = segb

            emit_loads(0)
            if NG > 1:
                emit_loads(1)
            emit_seg(0)

            for it in range(C + 3):
                c0, c1, c2, c3 = it, it - 1, it - 2, it - 3

                # prefetch
                if c0 % G == 0 and c0 // G + 2 < NG:
                    emit_loads(c0 // G + 2)
                if c0 % SG == 0 and c0 // SG + 1 <= (C - 1) // SG:
                    emit_seg(c0 // SG + 1)

                # ---- masks(c0): first on DVE (deps always ready)
                if c0 < C:
                    sg, si = c0 // SG, c0 % SG
                    segb = T_segb[sg]
                    a_s = pwk.tile([S, TC], F16, tag="a_s")
                    nc.vector.tensor_single_scalar(
                        out=a_s, in_=segb[:, si * TC : (si + 1) * TC],
                        scalar=iota_s_sb, op=eq,
                    )
                    a_t = pwk.tile([128, KT, S], F16, tag="a_t")
                    nc.vector.tensor_tensor(
                        out=a_t,
                        in0=bass.AP(tensor=iota_at_sb.tensor, offset=iota_at_sb.offset,
                                    ap=[list(iota_at_sb.ap[0]), [0, KT], [1, S]]),
                        in1=bass.AP(tensor=segc_sb.tensor,
                                    offset=segc_sb.offset + c0 * KT,
                                    ap=[list(segc_sb.ap[0]), [1, KT], [0, S]]),
                        op=eq,
                    )
                    T_as[c0] = a_s
                    T_at[c0] = a_t

                # ---- alpha(c3): matmuls into gate tile, then DVE scale+aat
                if 0 <= c3 < C:
                    gt3 = T_gt.pop(c3)
                    gb3 = T_al.pop(c3)       # gate psum tile of c3
                    alp = gb3[:, 0:KT]
                    for k in range(KT):
                        for h in range(2):
                            nc.tensor.matmul(
                                alp[:, k : k + 1],
                                gt3[:, h * TC + k * 128 : h * TC + (k + 1) * 128],
                                qwh_sb[:, h : h + 1],
                                start=(h == 0), stop=(h == 1),
                            )
                    alpha = pwk.tile([128, KT], F32, tag="alpha")
                    nc.vector.tensor_scalar(
                        out=alpha, in0=alp, scalar1=0.5, scalar2=qbp,
                        op0=mult, op1=add,
                    )
                    a_t3 = T_at.pop(c3)
                    aat = pwk.tile([128, KT, S], F16, tag="aat")
                    nc.vector.tensor_tensor(
                        out=aat,
                        in0=a_t3,
                        in1=bass.AP(tensor=alpha.tensor, offset=alpha.offset,
                                    ap=[list(alpha.ap[0]), [1, KT], [0, S]]),
                        op=mult,
                    )
                    T_aat[c3] = (gb3, aat)

                # ---- gate(c2) matmuls + tanh
                if 0 <= c2 < C:
                    g1_2 = T_g1.pop(c2)
                    a_s2 = T_as.pop(c2)
                    ph8_2 = T_ph8.pop(c2)
                    gp = pgt.tile([128, 2 * TC], F32, tag="gate")
                    for h in range(2):
                        dst = gp[:, h * TC : (h + 1) * TC]
                        nc.tensor.matmul(
                            dst, g1_2[:, h * 128 : (h + 1) * 128], a_s2,
                            start=True, stop=False,
                        )
                        nc.tensor.matmul(
                            dst,
                            w28_sb[:, :, h * 128 : (h + 1) * 128],
                            ph8_2.rearrange("p (i t) -> p i t", i=2),
                            start=False, stop=True, perf_mode=DR,
                        )
                    gt2 = pwk.tile([128, 2 * TC], F16, tag="gt")
                    nc.scalar.activation(out=gt2, in_=gp, func=Tanh, scale=0.5)
                    T_gt[c2] = gt2
                    T_al[c2] = gp

                # ---- ph(c0): DR matmuls + tanh -> fp8
                if c0 < C:
                    xt8_0 = T_xt8.pop(c0)
                    pf8_0 = T_pf8.pop(c0)
                    php = pph.tile([128, 2 * TC], F32, tag="ph")
                    for h in range(2):
                        dst = php[:, h * TC : (h + 1) * TC]
                        nc.tensor.matmul(
                            dst, wp18_sb[:, :, h * 128 : (h + 1) * 128], xt8_0,
                            start=True, stop=False, perf_mode=DR,
                        )
                        nc.tensor.matmul(
                            dst, id8_sb[:, :, h * 128 : (h + 1) * 128], pf8_0,
                            start=False, stop=True, perf_mode=DR,
                        )
                    ph8 = pwk.tile([128, 2 * TC], F8, tag="ph8")
                    nc.scalar.activation(out=ph8, in_=php, func=Tanh)
                    T_ph8[c0] = ph8

                # ---- ss(c0): transposed session sums + fp8 copy
                if c0 < C:
                    x16_0 = T_x16[c0]
                    a_t0 = T_at[c0]
                    ga = pga.tile([128, 2 * S + H], F32, tag="ga")
                    ss = ga[:, 0 : 2 * S]
                    for h in range(2):
                        for k in range(KT):
                            nc.tensor.matmul(
                                ss[:, h * S : (h + 1) * S],
                                x16_0[:, k, h * 128 : (h + 1) * 128],
                                a_t0[:, k, :],
                                start=(k == 0), stop=(k == KT - 1),
                            )
                    smt = pwk.tile([128, 2 * S], F8, tag="smt")
                    nc.vector.tensor_copy(out=smt, in_=ss)
                    T_smt[c0] = (smt, ga)

                # ---- g1(c1): DR matmul + scale/bias (late on PE so the
                # smt copy from last iteration has fully drained)
                if 0 <= c1 < C:
                    smt1, ga1 = T_smt.pop(c1)
                    g1p = ga1[0:S, 2 * S : 2 * S + H]
                    nc.tensor.matmul(
                        g1p,
                        smt1.rearrange("p (i s) -> p i s", i=2),
                        w18_sb[:],
                        start=True, stop=True, perf_mode=DR,
                    )
                    g1 = pwk.tile([S, H], F16, tag="g1")
                    nc.vector.scalar_tensor_tensor(
                        out=g1, in0=g1p, scalar=rec_sb[:, c1 : c1 + 1],
                        in1=bch_sb, op0=mult, op1=add,
                    )
                    T_g1[c1] = g1

                # ---- h_s(c3): transposed weighted sums, copy, store
                if 0 <= c3 < C:
                    gb3, aat3 = T_aat.pop(c3)
                    x16_3 = T_x16.pop(c3)
                    hsp = gb3[:, TC : TC + 2 * S]
                    for h in range(2):
                        for k in range(KT):
                            nc.tensor.matmul(
                                hsp[:, h * S : (h + 1) * S],
                                x16_3[:, k, h * 128 : (h + 1) * 128],
                                aat3[:, k, :],
                                start=(k == 0), stop=(k == KT - 1),
                            )
                    if c3 % G == 0:
                        T_hsg[c3 // G] = pwk.tile([128, G, 2 * S], F32, tag="hsg",
                                                  name="hsg")
                    hsg = T_hsg[c3 // G]
                    nc.vector.tensor_copy(out=hsg[:, c3 % G], in_=hsp)
                    if c3 % G == G - 1 or c3 == C - 1:
                        n = c3 % G + 1
                        lo = c3 - n + 1
                        nc.gpsimd.dma_start(
                            out=hst[:, lo : c3 + 1, :], in_=hsg[:, :n],
                        )
                        del T_hsg[c3 // G]

    _split_multi_waits(nc)
    return nc


_CACHE = {}


def kernel(hidden, pos_table, W_pos, b_pos, W1, b1, W2, b2, qw, qb,
           seq_len, reverse_pos):
    seq_len_np = np.asarray(seq_len)
    lens, core_chunks, C = _plan(seq_len_np)
    weights, qbp, pw8 = _pack_weights(
        pos_table, W_pos, b_pos, W1, b1, W2, b2, qw, qb
    )
    x16, xt8, pf8, seg_row, seg_col, recip, unpack_idx = _pack_inputs(
        hidden, reverse_pos, pw8, lens, core_chunks, C
    )

    key = (C, qbp)
    if key not in _CACHE:
        _CACHE[key] = _build_bass(C, qbp)
    nc = _CACHE[key]

    in_maps = []
    for core in range(N_CORES):
        m = dict(
            x16=x16[core], xt8=xt8[core], pf8=pf8[core],
            seg_row=seg_row[core], seg_col=seg_col[core], recip=recip[core],
        )
        m.update(weights)
        in_maps.append(m)

    import time as _time

    t0 = _time.perf_counter()
    res = run_bass_kernel_spmd(nc, in_maps, core_ids=list(range(N_CORES)))
    kernel._last_run_s = _time.perf_counter() - t0
    # hst: [N, 128, C, 2S] f32 -> h_s[sess, h] with h = 128*half + p
    hs_all = np.stack([res.results[i]["hst"] for i in range(N_CORES)])
    hs_all = hs_all.reshape(N_CORES, 128, C, 2, S)

    out_core, out_chunk, out_local = unpack_idx
    # [sess, half, p] -> [sess, 128*half + p]
    out = hs_all[out_core, :, out_chunk, :, out_local]      # [B, 128, 2]
    out = out.transpose(0, 2, 1).reshape(len(out_core), H)
    return np.ascontiguousarray(out)


# revision 23
# speedup vs baseline: 2.5467x; 1.0840x over previous
"""Trainium2 Bass kernel for the CNNFusing ragged-session attention pooling module.

Computes, per session s over its token set:
    v_mean   = mean(hidden[s])                                  [H]
    ph[t]    = tanh(hidden[t] @ Wp1 + (pos_table @ Wp2 + b_pos)[rp[t]])
    gate[t]  = sigmoid(v_mean @ W1 + b1 + ph[t] @ W2 + b2)
    alpha[t] = gate[t] @ qw + qb
    h_s      = sum_t alpha[t] * hidden[t]                       [B, H]

Data-parallel over sessions on 8 cores.  Sessions are FFD-bin-packed into
512-token chunks (<=32 sessions per chunk).  Ragged ops become one-hot
matmuls.  The two big H x H GEMM chains per token (ph pre-act and the gate
ph-term) run as fp8e4m3 DoubleRow matmuls (2 contraction rows/partition);
session sums / weighted sums stay fp16.  h_s is accumulated transposed
([h, s] layout) so each accumulation step costs S output rows instead of H.
The sigmoid is folded into tanh and both per-token biases ride the one-hot
mean-term matmul, so each stage needs a single full-width tanh activation.
"""

import numpy as np
import ml_dtypes

import concourse.bass as bass
import concourse.mybir as mybir
import concourse.tile as tile
from concourse.bass_utils import run_bass_kernel_spmd

H = 256
TC = 512      # tokens per chunk
S = 32        # max sessions per chunk
KT = TC // 128
G = 2         # chunks per batched load/store DMA
SG = 8        # chunks per seg-row broadcast DMA
N_CORES = 8

F8 = mybir.dt.float8e4
F16 = mybir.dt.float16
F32 = mybir.dt.float32
NP_F8 = ml_dtypes.float8_e4m3fn


# --------------------------------------------------------------------------
# The walrus build here accepts only ONE sync-wait command per instruction,
# while Tile may attach several.  Hoist all but the last wait of such
# instructions onto standalone event-semaphore waits inserted just before
# them on the same engine (sequencer executes in order, semantics kept).
_waitsplit_uid = [0]


def _split_multi_waits(nc):
    for fn in nc.m.functions:
        for bb in fn.blocks:
            insts = bb.instructions
            i = 0
            while i < len(insts):
                inst = insts[i]
                si = getattr(inst, "sync_info", None)
                waits = list(si.on_wait) if si is not None and si.on_wait else []
                if len(waits) > 1:
                    si.on_wait = waits[-1:]
                    for w in waits[:-1]:
                        ev = mybir.InstEventSemaphore(
                            name=f"I-waitsplit-{_waitsplit_uid[0]}", ins=[], outs=[]
                        )
                        _waitsplit_uid[0] += 1
                        ev.engine = inst.engine
                        ev.sync_info = mybir.SyncInfo(on_wait=[w], on_update=[])
                        insts.insert(i, ev)
                        i += 1
                i += 1
# --------------------------------------------------------------------------


def _plan(seq_len):
    """Best-fit-decreasing bin packing of all sessions into (token<=TC,
    sessions<=S) chunks, then deal chunks round-robin to cores."""
    lens = np.asarray(seq_len, dtype=np.int64)
    B = len(lens)
    order = np.argsort(-lens, kind="stable")
    bins = []          # list of [tok_used, [session ids]]
    # rem_sorted: sorted list of (remaining_tokens, bin_idx) for best-fit
    import bisect
    rem = []           # sorted (remaining, bin_idx)
    for sid in order:
        L = int(lens[sid])
        # best fit: smallest remaining >= L
        pos = bisect.bisect_left(rem, (L, -1))
        placed = False
        while pos < len(rem):
            r, bi = rem[pos]
            if len(bins[bi][1]) < S:
                rem.pop(pos)
                bins[bi][0] += L
                bins[bi][1].append(sid)
                nr = TC - bins[bi][0]
                if nr > 0:
                    bisect.insort(rem, (nr, bi))
                placed = True
                break
            pos += 1
        if not placed:
            bi = len(bins)
            bins.append([L, [sid]])
            bisect.insort(rem, (TC - L, bi))
    nb = len(bins)
    C = -(-nb // N_CORES)
    core_chunks = [[] for _ in range(N_CORES)]
    for i, b in enumerate(bins):
        core_chunks[i % N_CORES].append(b[1])
    return lens, core_chunks, C


def _pack_inputs(hidden, reverse_pos, pw8, lens, core_chunks, C):
    """Build all per-core DRAM input arrays."""
    B = len(lens)
    starts = np.concatenate([[0], np.cumsum(lens)[:-1]])
    hidden16 = np.asarray(hidden, np.float32).astype(np.float16)
    rp = np.asarray(reverse_pos)

    # token map [N, C, TC] -> global token index (or -1)
    tokmap = np.full((N_CORES, C, TC), -1, np.int64)
    seg_row = np.full((N_CORES, C, TC), -1.0, np.float16)
    recip = np.zeros((N_CORES, S, C), np.float32)
    out_core = np.zeros(B, np.int32)
    out_chunk = np.zeros(B, np.int32)
    out_local = np.zeros(B, np.int32)

    for core in range(N_CORES):
        for ci, sess in enumerate(core_chunks[core]):
            t = 0
            for si, sid in enumerate(sess):
                L = int(lens[sid])
                tokmap[core, ci, t : t + L] = np.arange(starts[sid], starts[sid] + L)
                seg_row[core, ci, t : t + L] = si
                recip[core, si, ci] = 1.0 / L
                out_core[sid] = core
                out_chunk[sid] = ci
                out_local[sid] = si
                t += L

    valid = tokmap >= 0
    idx = np.where(valid, tokmap, 0)

    # gathered hidden [N, C, TC, H] fp16 (zero padded)
    xt = hidden16[idx]
    xt[~valid] = 0
    # row tiles [N, C, 128, KT, H] f16
    x16 = np.ascontiguousarray(
        xt.reshape(N_CORES, C, KT, 128, H).transpose(0, 1, 3, 2, 4)
    )
    # transposed fp8 [N, C, 128, 2, TC]
    xt8 = np.ascontiguousarray(
        xt.astype(NP_F8).transpose(0, 1, 3, 2).reshape(N_CORES, C, 2, 128, TC)
        .transpose(0, 1, 3, 2, 4)
    )
    del xt

    # pos features (already fp8-quantized table), gathered transposed
    rpg = np.where(valid, rp[idx], 0)
    pft = pw8[rpg]                                  # [N, C, TC, H] fp8
    pft[~valid] = 0
    pf8 = np.ascontiguousarray(
        pft.transpose(0, 1, 3, 2).reshape(N_CORES, C, 2, 128, TC)
        .transpose(0, 1, 3, 2, 4)
    )
    del pft

    seg_col = np.ascontiguousarray(
        seg_row.reshape(N_CORES, C, KT, 128).transpose(0, 3, 1, 2)
    ).astype(np.float32)

    return x16, xt8, pf8, seg_row, seg_col, recip, (out_core, out_chunk, out_local)


def _pack_weights(pos_table, W_pos, b_pos, W1, b1, W2, b2, qw, qb):
    Wp = np.asarray(W_pos, np.float32)
    pwf = np.asarray(pos_table, np.float32) @ Wp[H:] + np.asarray(b_pos, np.float32)
    pw8 = np.zeros((H, H), NP_F8)
    pw8[: pwf.shape[0]] = pwf.astype(NP_F8)

    def pack_dr(M):  # [256, 256] -> [128, 2, 256] fp8, row c = 128*i + p
        return np.ascontiguousarray(
            np.asarray(M, np.float32).reshape(2, 128, H).transpose(1, 0, 2)
        ).astype(NP_F8)

    wp18 = pack_dr(Wp[:H])
    w28 = pack_dr(np.asarray(W2, np.float32))
    w18 = pack_dr(np.asarray(W1, np.float32))

    ident8 = np.zeros((128, 2, H), NP_F8)
    for m in range(2):
        ident8[:, m, m * 128 : (m + 1) * 128] = np.eye(128, dtype=NP_F8)

    qwf = np.asarray(qw, np.float32).reshape(H)
    qwh = np.ascontiguousarray(qwf.reshape(2, 128).T).astype(np.float16)
    qbp = float(np.asarray(qb, np.float32).reshape(()) + qwf.sum() / 2.0)
    # full (unscaled) bias b1+b2 rides the mean-term; ACT applies tanh(z/2)
    bcf = np.asarray(b1, np.float32) + np.asarray(b2, np.float32)
    bchrow = np.broadcast_to(bcf, (S, H)).copy().astype(np.float32)

    iota_at = np.broadcast_to(np.arange(S, dtype=np.float16), (128, S)).copy()
    wk8 = np.concatenate([wp18, ident8, w28, w18], axis=2)
    cf16 = np.concatenate([qwh, iota_at], axis=1).astype(np.float16)
    return dict(wk8=wk8, cf16=cf16, bchrow=bchrow), qbp, pw8


def _build_bass(C, qbp):
    nc = bass.Bass("TRN2", target_bir_lowering=False, debug=False,
                   num_devices=N_CORES)

    x16 = nc.dram_tensor("x16", [C, 128, KT, H], F16, kind="ExternalInput")
    xt8 = nc.dram_tensor("xt8", [C, 128, 2, TC], F8, kind="ExternalInput")
    pf8 = nc.dram_tensor("pf8", [C, 128, 2, TC], F8, kind="ExternalInput")
    seg_row = nc.dram_tensor("seg_row", [C, TC], F16, kind="ExternalInput")
    W32 = C * KT + H + 1 + C
    wk8 = nc.dram_tensor("wk8", [128, 2, 4 * H], F8, kind="ExternalInput")
    cf32 = nc.dram_tensor("cf32", [128, W32], F32, kind="ExternalInput")
    cf16 = nc.dram_tensor("cf16", [128, 2 + S], F16, kind="ExternalInput")
    hst = nc.dram_tensor("hst", [128, C, 2 * S], F32, kind="ExternalOutput")

    eq = mybir.AluOpType.is_equal
    mult = mybir.AluOpType.mult
    add = mybir.AluOpType.add
    Tanh = mybir.ActivationFunctionType.Tanh
    DR = mybir.MatmulPerfMode.DoubleRow

    NG = -(-C // G)    # number of load groups

    with tile.TileContext(nc) as tc:
        with (
            tc.tile_pool(name="consts", bufs=1) as pc,
            tc.tile_pool(name="loads", bufs=6) as pl,
            tc.tile_pool(name="segp", bufs=3) as psg,
            tc.tile_pool(name="work", bufs=10) as pwk,
            # PSUM: ph 1x2 banks + ga 2x1 + gate 2x2 = 8 banks
            tc.tile_pool(name="pph", bufs=1, space="PSUM") as pph,
            tc.tile_pool(name="pga", bufs=2, space="PSUM") as pga,
            tc.tile_pool(name="pgt", bufs=2, space="PSUM") as pgt,
        ):
            # ---- constants: 3 packed DMAs keep startup short ----
            wk8_sb = pc.tile([128, 2, 4 * H], F8)
            nc.sync.dma_start(out=wk8_sb, in_=wk8[:])
            cf16_sb = pc.tile([128, 2 + S], F16)
            nc.sync.dma_start(out=cf16_sb, in_=cf16[:])
            cf32_sb = pc.tile([128, W32], F32)
            nc.sync.dma_start(out=cf32_sb, in_=cf32[:])
            wp18_sb = wk8_sb[:, :, 0 * H : 1 * H]
            id8_sb = wk8_sb[:, :, 1 * H : 2 * H]
            w28_sb = wk8_sb[:, :, 2 * H : 3 * H]
            w18_sb = wk8_sb[:, :, 3 * H : 4 * H]
            qwh_sb = cf16_sb[:, 0:2]
            iota_at_sb = cf16_sb[:, 2 : 2 + S]
            segc_sb = cf32_sb[:, 0 : C * KT].rearrange("p (c k) -> p c k", c=C)
            bch_sb = cf32_sb[0:S, C * KT : C * KT + H]
            iota_s_sb = cf32_sb[0:S, C * KT + H : C * KT + H + 1]
            rec_sb = cf32_sb[0:S, C * KT + H + 1 : C * KT + H + 1 + C]

            T_x16, T_xt8, T_pf8 = {}, {}, {}
            T_segb = {}
            T_as, T_at, T_ph8, T_g1, T_smt, T_gt, T_al, T_aat = (
                {}, {}, {}, {}, {}, {}, {}, {}
            )
            T_hsg = {}

            def emit_loads(g):
                c = g * G
                ng = min(G, C - c)
                xt8_t = pl.tile([128, G, 2, TC], F8, tag="xt8")
                nc.sync.dma_start(
                    out=xt8_t[:, :ng], in_=xt8[c : c + ng].rearrange("c p i t -> p c i t")
                )
                pf8_t = pl.tile([128, G, 2, TC], F8, tag="pf8")
                nc.sync.dma_start(
                    out=pf8_t[:, :ng], in_=pf8[c : c + ng].rearrange("c p i t -> p c i t")
                )
                x16_t = pl.tile([128, G, KT, H], F16, tag="x16")
                nc.sync.dma_start(
                    out=x16_t[:, :ng], in_=x16[c : c + ng].rearrange("c p k h -> p c k h")
                )
                for j in range(ng):
                    T_x16[c + j] = x16_t[:, j]
                    T_xt8[c + j] = xt8_t[:, j]
                    T_pf8[c + j] = pf8_t[:, j]

            def emit_seg(sg):
                c = sg * SG
                n = min(SG, C - c)
                src = seg_row[c]
                segb = psg.tile([S, SG * TC], F16, tag="segb")
                nc.sync.dma_start(
                    out=segb[:, : n * TC],
                    in_=bass.AP(tensor=src.tensor, offset=src.offset,
                                ap=[[0, S], [1, n * TC]]),
                )
                T_segb[sg] = segb

            emit_loads(0)
            if NG > 1:
                emit_loads(1)
            emit_seg(0)

            for it in range(C + 3):
                c0, c1, c2, c3 = it, it - 1, it - 2, it - 3

                # prefetch
                if c0 % G == 0 and c0 // G + 2 < NG:
                    emit_loads(c0 // G + 2)
                if c0 % SG == 0 and c0 // SG + 1 <= (C - 1) // SG:
                    emit_seg(c0 // SG + 1)

                # ---- masks(c0): first on DVE (deps always ready)
                if c0 < C:
                    sg, si = c0 // SG, c0 % SG
                    segb = T_segb[sg]
                    a_s = pwk.tile([S, TC], F16, tag="a_s")
                    nc.vector.tensor_single_scalar(
                        out=a_s, in_=segb[:, si * TC : (si + 1) * TC],
                        scalar=iota_s_sb, op=eq,
                    )
                    a_t = pwk.tile([128, KT, S], F16, tag="a_t")
                    nc.vector.tensor_tensor(
                        out=a_t,
                        in0=bass.AP(tensor=iota_at_sb.tensor, offset=iota_at_sb.offset,
                                    ap=[list(iota_at_sb.ap[0]), [0, KT], [1, S]]),
                        in1=bass.AP(tensor=segc_sb.tensor,
                                    offset=segc_sb.offset + c0 * KT,
                                    ap=[list(segc_sb.ap[0]), [1, KT], [0, S]]),
                        op=eq,
                    )
                    T_as[c0] = a_s
                    T_at[c0] = a_t

                # ---- alpha(c3): matmuls into gate tile, then DVE scale+aat
                if 0 <= c3 < C:
                    gt3 = T_gt.pop(c3)
                    gb3 = T_al.pop(c3)       # gate psum tile of c3
                    alp = gb3[:, 0:KT]
                    for k in range(KT):
                        for h in range(2):
                            nc.tensor.matmul(
                                alp[:, k : k + 1],
                                gt3[:, h * TC + k * 128 : h * TC + (k + 1) * 128],
                                qwh_sb[:, h : h + 1],
                                start=(h == 0), stop=(h == 1),
                            )
                    alpha = pwk.tile([128, KT], F32, tag="alpha")
                    nc.vector.tensor_scalar(
                        out=alpha, in0=alp, scalar1=0.5, scalar2=qbp,
                        op0=mult, op1=add,
                    )
                    a_t3 = T_at.pop(c3)
                    aat = pwk.tile([128, KT, S], F16, tag="aat")
                    nc.vector.tensor_tensor(
                        out=aat,
                        in0=a_t3,
                        in1=bass.AP(tensor=alpha.tensor, offset=alpha.offset,
                                    ap=[list(alpha.ap[0]), [1, KT], [0, S]]),
                        op=mult,
                    )
                    T_aat[c3] = (gb3, aat)

                # ---- gate(c2) matmuls + tanh
                if 0 <= c2 < C:
                    g1_2 = T_g1.pop(c2)
                    a_s2 = T_as.pop(c2)
                    ph8_2 = T_ph8.pop(c2)
                    gp = pgt.tile([128, 2 * TC], F32, tag="gate")
                    for h in range(2):
                        dst = gp[:, h * TC : (h + 1) * TC]
                        nc.tensor.matmul(
                            dst, g1_2[:, h * 128 : (h + 1) * 128], a_s2,
                            start=True, stop=False,
                        )
                        nc.tensor.matmul(
                            dst,
                            w28_sb[:, :, h * 128 : (h + 1) * 128],
                            ph8_2.rearrange("p (i t) -> p i t", i=2),
                            start=False, stop=True, perf_mode=DR,
                        )
                    gt2 = pwk.tile([128, 2 * TC], F16, tag="gt")
                    nc.scalar.activation(out=gt2, in_=gp, func=Tanh, scale=0.5)
                    T_gt[c2] = gt2
                    T_al[c2] = gp

                # ---- ph(c0): DR matmuls + tanh -> fp8
                if c0 < C:
                    xt8_0 = T_xt8.pop(c0)
                    pf8_0 = T_pf8.pop(c0)
                    php = pph.tile([128, 2 * TC], F32, tag="ph")
                    for h in range(2):
                        dst = php[:, h * TC : (h + 1) * TC]
                        nc.tensor.matmul(
                            dst, wp18_sb[:, :, h * 128 : (h + 1) * 128], xt8_0,
                            start=True, stop=False, perf_mode=DR,
                        )
                        nc.tensor.matmul(
                            dst, id8_sb[:, :, h * 128 : (h + 1) * 128], pf8_0,
                            start=False, stop=True, perf_mode=DR,
                        )
                    ph8 = pwk.tile([128, 2 * TC], F8, tag="ph8")
                    nc.scalar.activation(out=ph8, in_=php, func=Tanh)
                    T_ph8[c0] = ph8

                # ---- ss(c0): transposed session sums + fp8 copy
                if c0 < C:
                    x16_0 = T_x16[c0]
                    a_t0 = T_at[c0]
                    ga = pga.tile([128, 2 * S + H], F32, tag="ga")
                    ss = ga[:, 0 : 2 * S]
                    for h in range(2):
                        for k in range(KT):
                            nc.tensor.matmul(
                                ss[:, h * S : (h + 1) * S],
                                x16_0[:, k, h * 128 : (h + 1) * 128],
                                a_t0[:, k, :],
                                start=(k == 0), stop=(k == KT - 1),
                            )
                    smt = pwk.tile([128, 2 * S], F8, tag="smt")
                    nc.vector.tensor_copy(out=smt, in_=ss)
                    T_smt[c0] = (smt, ga)

                # ---- g1(c1): DR matmul + scale/bias (late on PE so the
                # smt copy from last iteration has fully drained)
                if 0 <= c1 < C:
                    smt1, ga1 = T_smt.pop(c1)
                    g1p = ga1[0:S, 2 * S : 2 * S + H]
                    nc.tensor.matmul(
                        g1p,
                        smt1.rearrange("p (i s) -> p i s", i=2),
                        w18_sb[:],
                        start=True, stop=True, perf_mode=DR,
                    )
                    g1 = pwk.tile([S, H], F16, tag="g1")
                    nc.vector.scalar_tensor_tensor(
                        out=g1, in0=g1p, scalar=rec_sb[:, c1 : c1 + 1],
                        in1=bch_sb, op0=mult, op1=add,
                    )
                    T_g1[c1] = g1

                # ---- h_s(c3): transposed weighted sums, copy, store
                if 0 <= c3 < C:
                    gb3, aat3 = T_aat.pop(c3)
                    x16_3 = T_x16.pop(c3)
                    hsp = gb3[:, TC : TC + 2 * S]
                    for h in range(2):
                        for k in range(KT):
                            nc.tensor.matmul(
                                hsp[:, h * S : (h + 1) * S],
                                x16_3[:, k, h * 128 : (h + 1) * 128],
                                aat3[:, k, :],
                                start=(k == 0), stop=(k == KT - 1),
                            )
                    if c3 % G == 0:
                        T_hsg[c3 // G] = pwk.tile([128, G, 2 * S], F32, tag="hsg",
                                                  name="hsg")
                    hsg = T_hsg[c3 // G]
                    nc.vector.tensor_copy(out=hsg[:, c3 % G], in_=hsp)
                    if c3 % G == G - 1 or c3 == C - 1:
                        n = c3 % G + 1
                        lo = c3 - n + 1
                        nc.gpsimd.dma_start(
                            out=hst[:, lo : c3 + 1, :], in_=hsg[:, :n],
                        )
                        del T_hsg[c3 // G]

    _split_multi_waits(nc)
    return nc


_CACHE = {}


def kernel(hidden, pos_table, W_pos, b_pos, W1, b1, W2, b2, qw, qb,
           seq_len, reverse_pos):
    seq_len_np = np.asarray(seq_len)
    lens, core_chunks, C = _plan(seq_len_np)
    weights, qbp, pw8 = _pack_weights(
        pos_table, W_pos, b_pos, W1, b1, W2, b2, qw, qb
    )
    x16, xt8, pf8, seg_row, seg_col, recip, unpack_idx = _pack_inputs(
        hidden, reverse_pos, pw8, lens, core_chunks, C
    )

    key = (C, qbp)
    if key not in _CACHE:
        _CACHE[key] = _build_bass(C, qbp)
    nc = _CACHE[key]

    CKT = C * KT
    W32 = CKT + H + 1 + C
    in_maps = []
    for core in range(N_CORES):
        cf32 = np.zeros((128, W32), np.float32)
        cf32[:, :CKT] = seg_col[core].reshape(128, CKT)
        cf32[:S, CKT : CKT + H] = weights["bchrow"]
        cf32[:S, CKT + H] = np.arange(S, dtype=np.float32)
        cf32[:S, CKT + H + 1 :] = recip[core]
        m = dict(
            x16=x16[core], xt8=xt8[core], pf8=pf8[core],
            seg_row=seg_row[core], cf32=cf32,
            wk8=weights["wk8"], cf16=weights["cf16"],
        )
        in_maps.append(m)

    import time as _time

    t0 = _time.perf_counter()
    res = run_bass_kernel_spmd(nc, in_maps, core_ids=list(range(N_CORES)))
    kernel._last_run_s = _time.perf_counter() - t0
    # hst: [N, 128, C, 2S] f32 -> h_s[sess, h] with h = 128*half + p
    hs_all = np.stack([res.results[i]["hst"] for i in range(N_CORES)])
    hs_all = hs_all.reshape(N_CORES, 128, C, 2, S)

    out_core, out_chunk, out_local = unpack_idx
    # [sess, half, p] -> [sess, 128*half + p]
    out = hs_all[out_core, :, out_chunk, :, out_local]      # [B, 128, 2]
    out = out.transpose(0, 2, 1).reshape(len(out_core), H)
    return np.ascontiguousarray(out)


# revision 30
# speedup vs baseline: 2.5638x; 1.0067x over previous
"""Trainium2 Bass kernel for the CNNFusing ragged-session attention pooling module.

Computes, per session s over its token set:
    v_mean   = mean(hidden[s])                                  [H]
    ph[t]    = tanh(hidden[t] @ Wp1 + (pos_table @ Wp2 + b_pos)[rp[t]])
    gate[t]  = sigmoid(v_mean @ W1 + b1 + ph[t] @ W2 + b2)
    alpha[t] = gate[t] @ qw + qb
    h_s      = sum_t alpha[t] * hidden[t]                       [B, H]

Data-parallel over sessions on 8 cores.  Sessions are FFD-bin-packed into
512-token chunks (<=32 sessions per chunk).  Ragged ops become one-hot
matmuls.  The two big H x H GEMM chains per token (ph pre-act and the gate
ph-term) run as fp8e4m3 DoubleRow matmuls (2 contraction rows/partition);
session sums / weighted sums stay fp16.  h_s is accumulated transposed
([h, s] layout) so each accumulation step costs S output rows instead of H.
The sigmoid is folded into tanh and both per-token biases ride the one-hot
mean-term matmul, so each stage needs a single full-width tanh activation.
"""

import numpy as np
import ml_dtypes

import concourse.bass as bass
import concourse.mybir as mybir
import concourse.tile as tile
from concourse.bass_utils import run_bass_kernel_spmd

H = 256
TC = 512      # tokens per chunk
S = 32        # max sessions per chunk
KT = TC // 128
G = 2         # chunks per batched load DMA
SG = 8      # chunks per seg-row broadcast DMA
GST = 8     # chunks per batched store
N_CORES = 8

F8 = mybir.dt.float8e4
F16 = mybir.dt.float16
F32 = mybir.dt.float32
NP_F8 = ml_dtypes.float8_e4m3fn


# --------------------------------------------------------------------------
# The walrus build here accepts only ONE sync-wait command per instruction,
# while Tile may attach several.  Hoist all but the last wait of such
# instructions onto standalone event-semaphore waits inserted just before
# them on the same engine (sequencer executes in order, semantics kept).
_waitsplit_uid = [0]


def _split_multi_waits(nc):
    for fn in nc.m.functions:
        for bb in fn.blocks:
            insts = bb.instructions
            i = 0
            while i < len(insts):
                inst = insts[i]
                si = getattr(inst, "sync_info", None)
                waits = list(si.on_wait) if si is not None and si.on_wait else []
                if len(waits) > 1:
                    si.on_wait = waits[-1:]
                    for w in waits[:-1]:
                        ev = mybir.InstEventSemaphore(
                            name=f"I-waitsplit-{_waitsplit_uid[0]}", ins=[], outs=[]
                        )
                        _waitsplit_uid[0] += 1
                        ev.engine = inst.engine
                        ev.sync_info = mybir.SyncInfo(on_wait=[w], on_update=[])
                        insts.insert(i, ev)
                        i += 1
                i += 1
# --------------------------------------------------------------------------


def _plan(seq_len):
    """Best-fit-decreasing bin packing of all sessions into (token<=TC,
    sessions<=S) chunks, then deal chunks round-robin to cores."""
    lens = np.asarray(seq_len, dtype=np.int64)
    B = len(lens)
    order = np.argsort(-lens, kind="stable")
    bins = []          # list of [tok_used, [session ids]]
    # rem_sorted: sorted list of (remaining_tokens, bin_idx) for best-fit
    import bisect
    rem = []           # sorted (remaining, bin_idx)
    for sid in order:
        L = int(lens[sid])
        # best fit: smallest remaining >= L
        pos = bisect.bisect_left(rem, (L, -1))
        placed = False
        while pos < len(rem):
            r, bi = rem[pos]
            if len(bins[bi][1]) < S:
                rem.pop(pos)
                bins[bi][0] += L
                bins[bi][1].append(sid)
                nr = TC - bins[bi][0]
                if nr > 0:
                    bisect.insort(rem, (nr, bi))
                placed = True
                break
            pos += 1
        if not placed:
            bi = len(bins)
            bins.append([L, [sid]])
            bisect.insort(rem, (TC - L, bi))
    nb = len(bins)
    C = -(-nb // N_CORES)
    core_chunks = [[] for _ in range(N_CORES)]
    for i, b in enumerate(bins):
        core_chunks[i % N_CORES].append(b[1])
    return lens, core_chunks, C


def _pack_inputs(hidden, reverse_pos, pw8, lens, core_chunks, C):
    """Build all per-core DRAM input arrays."""
    B = len(lens)
    starts = np.concatenate([[0], np.cumsum(lens)[:-1]])
    hidden16 = np.asarray(hidden, np.float32).astype(np.float16)
    rp = np.asarray(reverse_pos)

    # token map [N, C, TC] -> global token index (or -1)
    tokmap = np.full((N_CORES, C, TC), -1, np.int64)
    seg_row = np.full((N_CORES, C, TC), -1.0, np.float16)
    recip = np.zeros((N_CORES, S, C), np.float32)
    out_core = np.zeros(B, np.int32)
    out_chunk = np.zeros(B, np.int32)
    out_local = np.zeros(B, np.int32)

    for core in range(N_CORES):
        for ci, sess in enumerate(core_chunks[core]):
            t = 0
            for si, sid in enumerate(sess):
                L = int(lens[sid])
                tokmap[core, ci, t : t + L] = np.arange(starts[sid], starts[sid] + L)
                seg_row[core, ci, t : t + L] = si
                recip[core, si, ci] = 1.0 / L
                out_core[sid] = core
                out_chunk[sid] = ci
                out_local[sid] = si
                t += L

    valid = tokmap >= 0
    idx = np.where(valid, tokmap, 0)

    # gathered hidden [N, C, TC, H] fp16 (zero padded)
    xt = hidden16[idx]
    xt[~valid] = 0
    # row tiles [N, C, 128, KT, H] f16
    x16 = np.ascontiguousarray(
        xt.reshape(N_CORES, C, KT, 128, H).transpose(0, 1, 3, 2, 4)
    )
    # transposed fp8 [N, C, 128, 2, TC]
    xt8 = np.ascontiguousarray(
        xt.astype(NP_F8).transpose(0, 1, 3, 2).reshape(N_CORES, C, 2, 128, TC)
        .transpose(0, 1, 3, 2, 4)
    )
    del xt

    # pos features (already fp8-quantized table), gathered transposed
    rpg = np.where(valid, rp[idx], 0)
    pft = pw8[rpg]                                  # [N, C, TC, H] fp8
    pft[~valid] = 0
    pf8 = np.ascontiguousarray(
        pft.transpose(0, 1, 3, 2).reshape(N_CORES, C, 2, 128, TC)
        .transpose(0, 1, 3, 2, 4)
    )
    del pft

    seg_col = np.ascontiguousarray(
        seg_row.reshape(N_CORES, C, KT, 128).transpose(0, 3, 1, 2)
    ).astype(np.float32)

    return x16, xt8, pf8, seg_row, seg_col, recip, (out_core, out_chunk, out_local)


def _pack_weights(pos_table, W_pos, b_pos, W1, b1, W2, b2, qw, qb):
    Wp = np.asarray(W_pos, np.float32)
    pwf = np.asarray(pos_table, np.float32) @ Wp[H:] + np.asarray(b_pos, np.float32)
    pw8 = np.zeros((H, H), NP_F8)
    pw8[: pwf.shape[0]] = pwf.astype(NP_F8)

    def pack_dr(M):  # [256, 256] -> [128, 2, 256] fp8, row c = 128*i + p
        return np.ascontiguousarray(
            np.asarray(M, np.float32).reshape(2, 128, H).transpose(1, 0, 2)
        ).astype(NP_F8)

    wp18 = pack_dr(Wp[:H])
    w28 = pack_dr(np.asarray(W2, np.float32))
    w18 = pack_dr(np.asarray(W1, np.float32))

    ident8 = np.zeros((128, 2, H), NP_F8)
    for m in range(2):
        ident8[:, m, m * 128 : (m + 1) * 128] = np.eye(128, dtype=NP_F8)

    qwf = np.asarray(qw, np.float32).reshape(H)
    qwh = np.ascontiguousarray(qwf.reshape(2, 128).T).astype(np.float16)
    qbp = float(np.asarray(qb, np.float32).reshape(()) + qwf.sum() / 2.0)
    # full (unscaled) bias b1+b2 rides the mean-term; ACT applies tanh(z/2)
    bcf = np.asarray(b1, np.float32) + np.asarray(b2, np.float32)
    bchrow = np.broadcast_to(bcf, (S, H)).copy().astype(np.float32)

    iota_at = np.broadcast_to(np.arange(S, dtype=np.float16), (128, S)).copy()
    wk8 = np.concatenate([wp18, ident8, w28, w18], axis=2)
    cf16 = np.concatenate([qwh, iota_at], axis=1).astype(np.float16)
    return dict(wk8=wk8, cf16=cf16, bchrow=bchrow), qbp, pw8


def _build_bass(C, qbp):
    nc = bass.Bass("TRN2", target_bir_lowering=False, debug=False,
                   num_devices=N_CORES)

    x16 = nc.dram_tensor("x16", [C, 128, KT, H], F16, kind="ExternalInput")
    xt8 = nc.dram_tensor("xt8", [C, 128, 2, TC], F8, kind="ExternalInput")
    pf8 = nc.dram_tensor("pf8", [C, 128, 2, TC], F8, kind="ExternalInput")
    seg_row = nc.dram_tensor("seg_row", [C, TC], F16, kind="ExternalInput")
    W32 = C * KT + H + 1 + C
    wk8 = nc.dram_tensor("wk8", [128, 2, 4 * H], F8, kind="ExternalInput")
    cf32 = nc.dram_tensor("cf32", [128, W32], F32, kind="ExternalInput")
    cf16 = nc.dram_tensor("cf16", [128, 2 + S], F16, kind="ExternalInput")
    hst = nc.dram_tensor("hst", [128, C, 2 * S], F32, kind="ExternalOutput")

    eq = mybir.AluOpType.is_equal
    mult = mybir.AluOpType.mult
    add = mybir.AluOpType.add
    Tanh = mybir.ActivationFunctionType.Tanh
    DR = mybir.MatmulPerfMode.DoubleRow

    NG = -(-C // G)    # number of load groups

    with tile.TileContext(nc) as tc:
        with (
            tc.tile_pool(name="consts", bufs=1) as pc,
            tc.tile_pool(name="loads", bufs=6) as pl,
            tc.tile_pool(name="segp", bufs=3) as psg,
            tc.tile_pool(name="work", bufs=10) as pwk,
            # PSUM: ph 1x2 banks + ga 2x1 + gate 2x2 = 8 banks
            tc.tile_pool(name="pph", bufs=1, space="PSUM") as pph,
            tc.tile_pool(name="pga", bufs=2, space="PSUM") as pga,
            tc.tile_pool(name="pgt", bufs=2, space="PSUM") as pgt,
        ):
            # ---- constants: 3 packed DMAs keep startup short ----
            wk8_sb = pc.tile([128, 2, 4 * H], F8)
            nc.sync.dma_start(out=wk8_sb, in_=wk8[:])
            cf16_sb = pc.tile([128, 2 + S], F16)
            nc.sync.dma_start(out=cf16_sb, in_=cf16[:])
            cf32_sb = pc.tile([128, W32], F32)
            nc.sync.dma_start(out=cf32_sb, in_=cf32[:])
            wp18_sb = wk8_sb[:, :, 0 * H : 1 * H]
            id8_sb = wk8_sb[:, :, 1 * H : 2 * H]
            w28_sb = wk8_sb[:, :, 2 * H : 3 * H]
            w18_sb = wk8_sb[:, :, 3 * H : 4 * H]
            qwh_sb = cf16_sb[:, 0:2]
            iota_at_sb = cf16_sb[:, 2 : 2 + S]
            segc_sb = cf32_sb[:, 0 : C * KT].rearrange("p (c k) -> p c k", c=C)
            bch_sb = cf32_sb[0:S, C * KT : C * KT + H]
            iota_s_sb = cf32_sb[0:S, C * KT + H : C * KT + H + 1]
            rec_sb = cf32_sb[0:S, C * KT + H + 1 : C * KT + H + 1 + C]

            T_x16, T_xt8, T_pf8 = {}, {}, {}
            T_segb = {}
            T_as, T_at, T_ph8, T_g1, T_smt, T_gt, T_al, T_aat = (
                {}, {}, {}, {}, {}, {}, {}, {}
            )
            T_hsg = {}

            def emit_loads(g):
                c = g * G
                ng = min(G, C - c)
                xt8_t = pl.tile([128, G, 2, TC], F8, tag="xt8")
                nc.sync.dma_start(
                    out=xt8_t[:, :ng], in_=xt8[c : c + ng].rearrange("c p i t -> p c i t")
                )
                pf8_t = pl.tile([128, G, 2, TC], F8, tag="pf8")
                nc.sync.dma_start(
                    out=pf8_t[:, :ng], in_=pf8[c : c + ng].rearrange("c p i t -> p c i t")
                )
                x16_t = pl.tile([128, G, KT, H], F16, tag="x16")
                nc.sync.dma_start(
                    out=x16_t[:, :ng], in_=x16[c : c + ng].rearrange("c p k h -> p c k h")
                )
                for j in range(ng):
                    T_x16[c + j] = x16_t[:, j]
                    T_xt8[c + j] = xt8_t[:, j]
                    T_pf8[c + j] = pf8_t[:, j]

            def emit_seg(sg):
                c = sg * SG
                n = min(SG, C - c)
                src = seg_row[c]
                segb = psg.tile([S, SG * TC], F16, tag="segb")
                nc.sync.dma_start(
                    out=segb[:, : n * TC],
                    in_=bass.AP(tensor=src.tensor, offset=src.offset,
                                ap=[[0, S], [1, n * TC]]),
                )
                T_segb[sg] = segb

            emit_loads(0)
            if NG > 1:
                emit_loads(1)
            emit_seg(0)

            for it in range(C + 3):
                c0, c1, c2, c3 = it, it - 1, it - 2, it - 3

                # prefetch
                if c0 % G == 0 and c0 // G + 2 < NG:
                    emit_loads(c0 // G + 2)
                if c0 % SG == 0 and c0 // SG + 1 <= (C - 1) // SG:
                    emit_seg(c0 // SG + 1)

                # ---- masks(c0): first on DVE (deps always ready)
                if c0 < C:
                    sg, si = c0 // SG, c0 % SG
                    segb = T_segb[sg]
                    a_s = pwk.tile([S, TC], F16, tag="a_s")
                    nc.vector.tensor_single_scalar(
                        out=a_s, in_=segb[:, si * TC : (si + 1) * TC],
                        scalar=iota_s_sb, op=eq,
                    )
                    a_t = pwk.tile([128, KT, S], F16, tag="a_t")
                    nc.vector.tensor_tensor(
                        out=a_t,
                        in0=bass.AP(tensor=iota_at_sb.tensor, offset=iota_at_sb.offset,
                                    ap=[list(iota_at_sb.ap[0]), [0, KT], [1, S]]),
                        in1=bass.AP(tensor=segc_sb.tensor,
                                    offset=segc_sb.offset + c0 * KT,
                                    ap=[list(segc_sb.ap[0]), [1, KT], [0, S]]),
                        op=eq,
                    )
                    T_as[c0] = a_s
                    T_at[c0] = a_t

                # ---- alpha(c3): matmuls into gate tile, then DVE scale+aat
                if 0 <= c3 < C:
                    gt3 = T_gt.pop(c3)
                    gb3 = T_al.pop(c3)       # gate psum tile of c3
                    alp = gb3[:, 0:KT]
                    for k in range(KT):
                        for h in range(2):
                            nc.tensor.matmul(
                                alp[:, k : k + 1],
                                gt3[:, h * TC + k * 128 : h * TC + (k + 1) * 128],
                                qwh_sb[:, h : h + 1],
                                start=(h == 0), stop=(h == 1),
                            )
                    alpha = pwk.tile([128, KT], F32, tag="alpha")
                    nc.vector.tensor_scalar(
                        out=alpha, in0=alp, scalar1=0.5, scalar2=qbp,
                        op0=mult, op1=add,
                    )
                    a_t3 = T_at.pop(c3)
                    aat = pwk.tile([128, KT, S], F16, tag="aat")
                    nc.vector.tensor_tensor(
                        out=aat,
                        in0=a_t3,
                        in1=bass.AP(tensor=alpha.tensor, offset=alpha.offset,
                                    ap=[list(alpha.ap[0]), [1, KT], [0, S]]),
                        op=mult,
                    )
                    T_aat[c3] = (gb3, aat)

                # ---- gate(c2) matmuls + tanh
                if 0 <= c2 < C:
                    g1_2 = T_g1.pop(c2)
                    a_s2 = T_as.pop(c2)
                    ph8_2 = T_ph8.pop(c2)
                    gp = pgt.tile([128, 2 * TC], F32, tag="gate")
                    for h in range(2):
                        dst = gp[:, h * TC : (h + 1) * TC]
                        nc.tensor.matmul(
                            dst, g1_2[:, h * 128 : (h + 1) * 128], a_s2,
                            start=True, stop=False,
                        )
                        nc.tensor.matmul(
                            dst,
                            w28_sb[:, :, h * 128 : (h + 1) * 128],
                            ph8_2.rearrange("p (i t) -> p i t", i=2),
                            start=False, stop=True, perf_mode=DR,
                        )
                    gt2 = pwk.tile([128, 2 * TC], F16, tag="gt")
                    nc.scalar.activation(out=gt2, in_=gp, func=Tanh, scale=0.5)
                    T_gt[c2] = gt2
                    T_al[c2] = gp

                # ---- ph(c0): DR matmuls + tanh -> fp8
                if c0 < C:
                    xt8_0 = T_xt8.pop(c0)
                    pf8_0 = T_pf8.pop(c0)
                    php = pph.tile([128, 2 * TC], F32, tag="ph")
                    for h in range(2):
                        dst = php[:, h * TC : (h + 1) * TC]
                        nc.tensor.matmul(
                            dst, wp18_sb[:, :, h * 128 : (h + 1) * 128], xt8_0,
                            start=True, stop=False, perf_mode=DR,
                        )
                        nc.tensor.matmul(
                            dst, id8_sb[:, :, h * 128 : (h + 1) * 128], pf8_0,
                            start=False, stop=True, perf_mode=DR,
                        )
                    ph8 = pwk.tile([128, 2 * TC], F8, tag="ph8")
                    nc.scalar.activation(out=ph8, in_=php, func=Tanh)
                    T_ph8[c0] = ph8

                # ---- ss(c0): transposed session sums + fp8 copy
                if c0 < C:
                    x16_0 = T_x16[c0]
                    a_t0 = T_at[c0]
                    ga = pga.tile([128, 2 * S + H], F32, tag="ga")
                    ss = ga[:, 0 : 2 * S]
                    for h in range(2):
                        for k in range(KT):
                            nc.tensor.matmul(
                                ss[:, h * S : (h + 1) * S],
                                x16_0[:, k, h * 128 : (h + 1) * 128],
                                a_t0[:, k, :],
                                start=(k == 0), stop=(k == KT - 1),
                            )
                    smt = pwk.tile([128, 2 * S], F8, tag="smt")
                    nc.vector.tensor_copy(out=smt, in_=ss)
                    T_smt[c0] = (smt, ga)

                # ---- g1(c1): DR matmul + scale/bias (late on PE so the
                # smt copy from last iteration has fully drained)
                if 0 <= c1 < C:
                    smt1, ga1 = T_smt.pop(c1)
                    g1p = ga1[0:S, 2 * S : 2 * S + H]
                    nc.tensor.matmul(
                        g1p,
                        smt1.rearrange("p (i s) -> p i s", i=2),
                        w18_sb[:],
                        start=True, stop=True, perf_mode=DR,
                    )
                    g1 = pwk.tile([S, H], F16, tag="g1")
                    nc.vector.scalar_tensor_tensor(
                        out=g1, in0=g1p, scalar=rec_sb[:, c1 : c1 + 1],
                        in1=bch_sb, op0=mult, op1=add,
                    )
                    T_g1[c1] = g1

                # ---- h_s(c3): transposed weighted sums, copy, store
                if 0 <= c3 < C:
                    gb3, aat3 = T_aat.pop(c3)
                    x16_3 = T_x16.pop(c3)
                    hsp = gb3[:, TC : TC + 2 * S]
                    for h in range(2):
                        for k in range(KT):
                            nc.tensor.matmul(
                                hsp[:, h * S : (h + 1) * S],
                                x16_3[:, k, h * 128 : (h + 1) * 128],
                                aat3[:, k, :],
                                start=(k == 0), stop=(k == KT - 1),
                            )
                    if c3 % GST == 0:
                        T_hsg[c3 // GST] = pwk.tile([128, GST, 2 * S], F32, tag="hsg",
                                                  name="hsg")
                    hsg = T_hsg[c3 // GST]
                    nc.vector.tensor_copy(out=hsg[:, c3 % GST], in_=hsp)
                    if c3 % GST == GST - 1 or c3 == C - 1:
                        n = c3 % GST + 1
                        lo = c3 - n + 1
                        nc.gpsimd.dma_start(
                            out=hst[:, lo : c3 + 1, :], in_=hsg[:, :n],
                        )
                        del T_hsg[c3 // GST]

    _split_multi_waits(nc)
    return nc


_CACHE = {}


def kernel(hidden, pos_table, W_pos, b_pos, W1, b1, W2, b2, qw, qb,
           seq_len, reverse_pos):
    seq_len_np = np.asarray(seq_len)
    lens, core_chunks, C = _plan(seq_len_np)
    weights, qbp, pw8 = _pack_weights(
        pos_table, W_pos, b_pos, W1, b1, W2, b2, qw, qb
    )
    x16, xt8, pf8, seg_row, seg_col, recip, unpack_idx = _pack_inputs(
        hidden, reverse_pos, pw8, lens, core_chunks, C
    )

    key = (C, qbp)
    if key not in _CACHE:
        _CACHE[key] = _build_bass(C, qbp)
    nc = _CACHE[key]

    CKT = C * KT
    W32 = CKT + H + 1 + C
    in_maps = []
    for core in range(N_CORES):
        cf32 = np.zeros((128, W32), np.float32)
        cf32[:, :CKT] = seg_col[core].reshape(128, CKT)
        cf32[:S, CKT : CKT + H] = weights["bchrow"]
        cf32[:S, CKT + H] = np.arange(S, dtype=np.float32)
        cf32[:S, CKT + H + 1 :] = recip[core]
        m = dict(
            x16=x16[core], xt8=xt8[core], pf8=pf8[core],
            seg_row=seg_row[core], cf32=cf32,
            wk8=weights["wk8"], cf16=weights["cf16"],
        )
        in_maps.append(m)

    import time as _time

    t0 = _time.perf_counter()
    res = run_bass_kernel_spmd(nc, in_maps, core_ids=list(range(N_CORES)))
    kernel._last_run_s = _time.perf_counter() - t0
    # hst: [N, 128, C, 2S] f32 -> h_s[sess, h] with h = 128*half + p
    hs_all = np.stack([res.results[i]["hst"] for i in range(N_CORES)])
    hs_all = hs_all.reshape(N_CORES, 128, C, 2, S)

    out_core, out_chunk, out_local = unpack_idx
    # [sess, half, p] -> [sess, 128*half + p]
    out = hs_all[out_core, :, out_chunk, :, out_local]      # [B, 128, 2]
    out = out.transpose(0, 2, 1).reshape(len(out_core), H)
    return np.ascontiguousarray(out)


# revision 33
# speedup vs baseline: 2.5808x; 1.0066x over previous
"""Trainium2 Bass kernel for the CNNFusing ragged-session attention pooling module.

Computes, per session s over its token set:
    v_mean   = mean(hidden[s])                                  [H]
    ph[t]    = tanh(hidden[t] @ Wp1 + (pos_table @ Wp2 + b_pos)[rp[t]])
    gate[t]  = sigmoid(v_mean @ W1 + b1 + ph[t] @ W2 + b2)
    alpha[t] = gate[t] @ qw + qb
    h_s      = sum_t alpha[t] * hidden[t]                       [B, H]

Data-parallel over sessions on 8 cores.  Sessions are FFD-bin-packed into
512-token chunks (<=32 sessions per chunk).  Ragged ops become one-hot
matmuls.  The two big H x H GEMM chains per token (ph pre-act and the gate
ph-term) run as fp8e4m3 DoubleRow matmuls (2 contraction rows/partition);
session sums / weighted sums stay fp16.  h_s is accumulated transposed
([h, s] layout) so each accumulation step costs S output rows instead of H.
The sigmoid is folded into tanh and both per-token biases ride the one-hot
mean-term matmul, so each stage needs a single full-width tanh activation.
"""

import numpy as np
import ml_dtypes

import concourse.bass as bass
import concourse.mybir as mybir
import concourse.tile as tile
from concourse.bass_utils import run_bass_kernel_spmd

H = 256
TC = 512      # tokens per chunk
S = 32        # max sessions per chunk
KT = TC // 128
G = 2         # chunks per batched load DMA
SG = 8      # chunks per seg-row broadcast DMA
GST = 8     # chunks per batched store
N_CORES = 8

F8 = mybir.dt.float8e4
F16 = mybir.dt.float16
F32 = mybir.dt.float32
NP_F8 = ml_dtypes.float8_e4m3fn


# --------------------------------------------------------------------------
# The walrus build here accepts only ONE sync-wait command per instruction,
# while Tile may attach several.  Hoist all but the last wait of such
# instructions onto standalone event-semaphore waits inserted just before
# them on the same engine (sequencer executes in order, semantics kept).
_waitsplit_uid = [0]


def _split_multi_waits(nc):
    for fn in nc.m.functions:
        for bb in fn.blocks:
            insts = bb.instructions
            i = 0
            while i < len(insts):
                inst = insts[i]
                si = getattr(inst, "sync_info", None)
                waits = list(si.on_wait) if si is not None and si.on_wait else []
                if len(waits) > 1:
                    si.on_wait = waits[-1:]
                    for w in waits[:-1]:
                        ev = mybir.InstEventSemaphore(
                            name=f"I-waitsplit-{_waitsplit_uid[0]}", ins=[], outs=[]
                        )
                        _waitsplit_uid[0] += 1
                        ev.engine = inst.engine
                        ev.sync_info = mybir.SyncInfo(on_wait=[w], on_update=[])
                        insts.insert(i, ev)
                        i += 1
                i += 1
# --------------------------------------------------------------------------


def _plan(seq_len):
    """Best-fit-decreasing bin packing of all sessions into (token<=TC,
    sessions<=S) chunks, then deal chunks round-robin to cores."""
    lens = np.asarray(seq_len, dtype=np.int64)
    B = len(lens)
    order = np.argsort(-lens, kind="stable")
    bins = []          # list of [tok_used, [session ids]]
    # rem_sorted: sorted list of (remaining_tokens, bin_idx) for best-fit
    import bisect
    rem = []           # sorted (remaining, bin_idx)
    for sid in order:
        L = int(lens[sid])
        # best fit: smallest remaining >= L
        pos = bisect.bisect_left(rem, (L, -1))
        placed = False
        while pos < len(rem):
            r, bi = rem[pos]
            if len(bins[bi][1]) < S:
                rem.pop(pos)
                bins[bi][0] += L
                bins[bi][1].append(sid)
                nr = TC - bins[bi][0]
                if nr > 0:
                    bisect.insort(rem, (nr, bi))
                placed = True
                break
            pos += 1
        if not placed:
            bi = len(bins)
            bins.append([L, [sid]])
            bisect.insort(rem, (TC - L, bi))
    nb = len(bins)
    C = -(-nb // N_CORES)
    core_chunks = [[] for _ in range(N_CORES)]
    for i, b in enumerate(bins):
        core_chunks[i % N_CORES].append(b[1])
    return lens, core_chunks, C


def _pack_inputs(hidden, reverse_pos, pw8, lens, core_chunks, C):
    """Build all per-core DRAM input arrays."""
    B = len(lens)
    starts = np.concatenate([[0], np.cumsum(lens)[:-1]])
    hidden16 = np.asarray(hidden, np.float32).astype(np.float16)
    rp = np.asarray(reverse_pos)

    # token map [N, C, TC] -> global token index (or -1)
    tokmap = np.full((N_CORES, C, TC), -1, np.int64)
    seg_row = np.full((N_CORES, C, TC), -1.0, np.float16)
    recip = np.zeros((N_CORES, S, C), np.float32)
    out_core = np.zeros(B, np.int32)
    out_chunk = np.zeros(B, np.int32)
    out_local = np.zeros(B, np.int32)

    for core in range(N_CORES):
        for ci, sess in enumerate(core_chunks[core]):
            t = 0
            for si, sid in enumerate(sess):
                L = int(lens[sid])
                tokmap[core, ci, t : t + L] = np.arange(starts[sid], starts[sid] + L)
                seg_row[core, ci, t : t + L] = si
                recip[core, si, ci] = 1.0 / L
                out_core[sid] = core
                out_chunk[sid] = ci
                out_local[sid] = si
                t += L

    valid = tokmap >= 0
    idx = np.where(valid, tokmap, 0)

    # gathered hidden [N, C, TC, H] fp16 (zero padded)
    xt = hidden16[idx]
    xt[~valid] = 0
    # row tiles [N, C, 128, KT, H] f16
    x16 = np.ascontiguousarray(
        xt.reshape(N_CORES, C, KT, 128, H).transpose(0, 1, 3, 2, 4)
    )
    # transposed fp8 [N, C, 128, 2, TC]
    xt8 = np.ascontiguousarray(
        xt.astype(NP_F8).transpose(0, 1, 3, 2).reshape(N_CORES, C, 2, 128, TC)
        .transpose(0, 1, 3, 2, 4)
    )
    del xt

    # pos features (already fp8-quantized table), gathered transposed
    rpg = np.where(valid, rp[idx], 0)
    pft = pw8[rpg]                                  # [N, C, TC, H] fp8
    pft[~valid] = 0
    pf8 = np.ascontiguousarray(
        pft.transpose(0, 1, 3, 2).reshape(N_CORES, C, 2, 128, TC)
        .transpose(0, 1, 3, 2, 4)
    )
    del pft

    seg_col = np.ascontiguousarray(
        seg_row.reshape(N_CORES, C, KT, 128).transpose(0, 3, 1, 2)
    ).astype(np.float32)

    return x16, xt8, pf8, seg_row, seg_col, recip, (out_core, out_chunk, out_local)


def _pack_weights(pos_table, W_pos, b_pos, W1, b1, W2, b2, qw, qb):
    Wp = np.asarray(W_pos, np.float32)
    pwf = np.asarray(pos_table, np.float32) @ Wp[H:] + np.asarray(b_pos, np.float32)
    pw8 = np.zeros((H, H), NP_F8)
    pw8[: pwf.shape[0]] = pwf.astype(NP_F8)

    def pack_dr(M):  # [256, 256] -> [128, 2, 256] fp8, row c = 128*i + p
        return np.ascontiguousarray(
            np.asarray(M, np.float32).reshape(2, 128, H).transpose(1, 0, 2)
        ).astype(NP_F8)

    wp18 = pack_dr(Wp[:H])
    w28 = pack_dr(np.asarray(W2, np.float32))
    w18 = pack_dr(np.asarray(W1, np.float32))

    ident8 = np.zeros((128, 2, H), NP_F8)
    for m in range(2):
        ident8[:, m, m * 128 : (m + 1) * 128] = np.eye(128, dtype=NP_F8)

    qwf = np.asarray(qw, np.float32).reshape(H)
    qwh = np.ascontiguousarray(qwf.reshape(2, 128).T).astype(np.float16)
    qbp = float(np.asarray(qb, np.float32).reshape(()) + qwf.sum() / 2.0)
    # full (unscaled) bias b1+b2 rides the mean-term; ACT applies tanh(z/2)
    bcf = np.asarray(b1, np.float32) + np.asarray(b2, np.float32)
    bchrow = np.broadcast_to(bcf, (S, H)).copy().astype(np.float32)

    iota_at = np.broadcast_to(np.arange(S, dtype=np.float16), (128, S)).copy()
    wk8 = np.concatenate([wp18, ident8, w28, w18], axis=2)
    cf16 = np.concatenate([qwh, iota_at], axis=1).astype(np.float16)
    return dict(wk8=wk8, cf16=cf16, bchrow=bchrow), qbp, pw8


def _build_bass(C, qbp):
    nc = bass.Bass("TRN2", target_bir_lowering=False, debug=False,
                   num_devices=N_CORES)

    x16 = nc.dram_tensor("x16", [C, 128, KT, H], F16, kind="ExternalInput")
    xt8 = nc.dram_tensor("xt8", [C, 128, 2, TC], F8, kind="ExternalInput")
    pf8 = nc.dram_tensor("pf8", [C, 128, 2, TC], F8, kind="ExternalInput")
    seg_row = nc.dram_tensor("seg_row", [C, TC], F16, kind="ExternalInput")
    W32 = C * KT + H + 1 + C
    wk8 = nc.dram_tensor("wk8", [128, 2, 4 * H], F8, kind="ExternalInput")
    cf32 = nc.dram_tensor("cf32", [128, W32], F32, kind="ExternalInput")
    cf16 = nc.dram_tensor("cf16", [128, 2 + S], F16, kind="ExternalInput")
    hst = nc.dram_tensor("hst", [128, C, 2 * S], F32, kind="ExternalOutput")

    eq = mybir.AluOpType.is_equal
    mult = mybir.AluOpType.mult
    add = mybir.AluOpType.add
    Tanh = mybir.ActivationFunctionType.Tanh
    DR = mybir.MatmulPerfMode.DoubleRow

    NG = -(-C // G)    # number of load groups

    with tile.TileContext(nc) as tc:
        with (
            tc.tile_pool(name="consts", bufs=1) as pc,
            tc.tile_pool(name="loads", bufs=6) as pl,
            tc.tile_pool(name="segp", bufs=3) as psg,
            tc.tile_pool(name="work", bufs=10) as pwk,
            # PSUM: ph 1x2 banks + ga 2x1 + gate 2x2 = 8 banks
            tc.tile_pool(name="pph", bufs=1, space="PSUM") as pph,
            tc.tile_pool(name="pga", bufs=2, space="PSUM") as pga,
            tc.tile_pool(name="pgt", bufs=2, space="PSUM") as pgt,
        ):
            # ---- constants: 3 packed DMAs keep startup short ----
            wk8_sb = pc.tile([128, 2, 4 * H], F8)
            nc.sync.dma_start(out=wk8_sb, in_=wk8[:])
            cf16_sb = pc.tile([128, 2 + S], F16)
            cf32_sb = pc.tile([128, W32], F32)
            wp18_sb = wk8_sb[:, :, 0 * H : 1 * H]
            id8_sb = wk8_sb[:, :, 1 * H : 2 * H]
            w28_sb = wk8_sb[:, :, 2 * H : 3 * H]
            w18_sb = wk8_sb[:, :, 3 * H : 4 * H]
            qwh_sb = cf16_sb[:, 0:2]
            iota_at_sb = cf16_sb[:, 2 : 2 + S]
            segc_sb = cf32_sb[:, 0 : C * KT].rearrange("p (c k) -> p c k", c=C)
            bch_sb = cf32_sb[0:S, C * KT : C * KT + H]
            iota_s_sb = cf32_sb[0:S, C * KT + H : C * KT + H + 1]
            rec_sb = cf32_sb[0:S, C * KT + H + 1 : C * KT + H + 1 + C]

            T_x16, T_xt8, T_pf8 = {}, {}, {}
            T_segb = {}
            T_as, T_at, T_ph8, T_g1, T_smt, T_gt, T_al, T_aat = (
                {}, {}, {}, {}, {}, {}, {}, {}
            )
            T_hsg = {}

            def emit_loads(g, part=None):
                c = g * G
                ng = min(G, C - c)
                if part in (None, 0):
                    xt8_t = pl.tile([128, G, 2, TC], F8, tag="xt8")
                    nc.sync.dma_start(
                        out=xt8_t[:, :ng],
                        in_=xt8[c : c + ng].rearrange("c p i t -> p c i t"),
                    )
                    pf8_t = pl.tile([128, G, 2, TC], F8, tag="pf8")
                    nc.sync.dma_start(
                        out=pf8_t[:, :ng],
                        in_=pf8[c : c + ng].rearrange("c p i t -> p c i t"),
                    )
                    for j in range(ng):
                        T_xt8[c + j] = xt8_t[:, j]
                        T_pf8[c + j] = pf8_t[:, j]
                if part in (None, 1):
                    x16_t = pl.tile([128, G, KT, H], F16, tag="x16")
                    nc.sync.dma_start(
                        out=x16_t[:, :ng],
                        in_=x16[c : c + ng].rearrange("c p k h -> p c k h"),
                    )
                    for j in range(ng):
                        T_x16[c + j] = x16_t[:, j]

            def emit_seg(sg):
                c = sg * SG
                n = min(SG, C - c)
                src = seg_row[c]
                segb = psg.tile([S, SG * TC], F16, tag="segb")
                nc.sync.dma_start(
                    out=segb[:, : n * TC],
                    in_=bass.AP(tensor=src.tensor, offset=src.offset,
                                ap=[[0, S], [1, n * TC]]),
                )
                T_segb[sg] = segb

            emit_loads(0, part=0)
            nc.sync.dma_start(out=cf16_sb, in_=cf16[:])
            nc.sync.dma_start(out=cf32_sb, in_=cf32[:])
            emit_seg(0)
            emit_loads(0, part=1)
            if NG > 1:
                emit_loads(1)

            for it in range(C + 3):
                c0, c1, c2, c3 = it, it - 1, it - 2, it - 3

                # prefetch
                if c0 % G == 0 and c0 // G + 2 < NG:
                    emit_loads(c0 // G + 2)
                if c0 % SG == 0 and c0 // SG + 1 <= (C - 1) // SG:
                    emit_seg(c0 // SG + 1)

                # ---- masks(c0): first on DVE (deps always ready)
                if c0 < C:
                    sg, si = c0 // SG, c0 % SG
                    segb = T_segb[sg]
                    a_s = pwk.tile([S, TC], F16, tag="a_s")
                    nc.vector.tensor_single_scalar(
                        out=a_s, in_=segb[:, si * TC : (si + 1) * TC],
                        scalar=iota_s_sb, op=eq,
                    )
                    a_t = pwk.tile([128, KT, S], F16, tag="a_t")
                    nc.vector.tensor_tensor(
                        out=a_t,
                        in0=bass.AP(tensor=iota_at_sb.tensor, offset=iota_at_sb.offset,
                                    ap=[list(iota_at_sb.ap[0]), [0, KT], [1, S]]),
                        in1=bass.AP(tensor=segc_sb.tensor,
                                    offset=segc_sb.offset + c0 * KT,
                                    ap=[list(segc_sb.ap[0]), [1, KT], [0, S]]),
                        op=eq,
                    )
                    T_as[c0] = a_s
                    T_at[c0] = a_t

                # ---- alpha(c3): matmuls into gate tile, then DVE scale+aat
                if 0 <= c3 < C:
                    gt3 = T_gt.pop(c3)
                    gb3 = T_al.pop(c3)       # gate psum tile of c3
                    alp = gb3[:, 0:KT]
                    for k in range(KT):
                        for h in range(2):
                            nc.tensor.matmul(
                                alp[:, k : k + 1],
                                gt3[:, h * TC + k * 128 : h * TC + (k + 1) * 128],
                                qwh_sb[:, h : h + 1],
                                start=(h == 0), stop=(h == 1),
                            )
                    alpha = pwk.tile([128, KT], F32, tag="alpha")
                    nc.vector.tensor_scalar(
                        out=alpha, in0=alp, scalar1=0.5, scalar2=qbp,
                        op0=mult, op1=add,
                    )
                    a_t3 = T_at.pop(c3)
                    aat = pwk.tile([128, KT, S], F16, tag="aat")
                    nc.vector.tensor_tensor(
                        out=aat,
                        in0=a_t3,
                        in1=bass.AP(tensor=alpha.tensor, offset=alpha.offset,
                                    ap=[list(alpha.ap[0]), [1, KT], [0, S]]),
                        op=mult,
                    )
                    T_aat[c3] = (gb3, aat)

                # ---- gate(c2) matmuls + tanh
                if 0 <= c2 < C:
                    g1_2 = T_g1.pop(c2)
                    a_s2 = T_as.pop(c2)
                    ph8_2 = T_ph8.pop(c2)
                    gp = pgt.tile([128, 2 * TC], F32, tag="gate")
                    for h in range(2):
                        dst = gp[:, h * TC : (h + 1) * TC]
                        nc.tensor.matmul(
                            dst, g1_2[:, h * 128 : (h + 1) * 128], a_s2,
                            start=True, stop=False,
                        )
                        nc.tensor.matmul(
                            dst,
                            w28_sb[:, :, h * 128 : (h + 1) * 128],
                            ph8_2.rearrange("p (i t) -> p i t", i=2),
                            start=False, stop=True, perf_mode=DR,
                        )
                    gt2 = pwk.tile([128, 2 * TC], F16, tag="gt")
                    nc.scalar.activation(out=gt2, in_=gp, func=Tanh, scale=0.5)
                    T_gt[c2] = gt2
                    T_al[c2] = gp

                # ---- ph(c0): DR matmuls + tanh -> fp8
                if c0 < C:
                    xt8_0 = T_xt8.pop(c0)
                    pf8_0 = T_pf8.pop(c0)
                    php = pph.tile([128, 2 * TC], F32, tag="ph")
                    for h in range(2):
                        dst = php[:, h * TC : (h + 1) * TC]
                        nc.tensor.matmul(
                            dst, wp18_sb[:, :, h * 128 : (h + 1) * 128], xt8_0,
                            start=True, stop=False, perf_mode=DR,
                        )
                        nc.tensor.matmul(
                            dst, id8_sb[:, :, h * 128 : (h + 1) * 128], pf8_0,
                            start=False, stop=True, perf_mode=DR,
                        )
                    ph8 = pwk.tile([128, 2 * TC], F8, tag="ph8")
                    nc.scalar.activation(out=ph8, in_=php, func=Tanh)
                    T_ph8[c0] = ph8

                # ---- ss(c0): transposed session sums + fp8 copy
                if c0 < C:
                    x16_0 = T_x16[c0]
                    a_t0 = T_at[c0]
                    ga = pga.tile([128, 2 * S + H], F32, tag="ga")
                    ss = ga[:, 0 : 2 * S]
                    for h in range(2):
                        for k in range(KT):
                            nc.tensor.matmul(
                                ss[:, h * S : (h + 1) * S],
                                x16_0[:, k, h * 128 : (h + 1) * 128],
                                a_t0[:, k, :],
                                start=(k == 0), stop=(k == KT - 1),
                            )
                    smt = pwk.tile([128, 2 * S], F8, tag="smt")
                    nc.vector.tensor_copy(out=smt, in_=ss)
                    T_smt[c0] = (smt, ga)

                # ---- g1(c1): DR matmul + scale/bias (late on PE so the
                # smt copy from last iteration has fully drained)
                if 0 <= c1 < C:
                    smt1, ga1 = T_smt.pop(c1)
                    g1p = ga1[0:S, 2 * S : 2 * S + H]
                    nc.tensor.matmul(
                        g1p,
                        smt1.rearrange("p (i s) -> p i s", i=2),
                        w18_sb[:],
                        start=True, stop=True, perf_mode=DR,
                    )
                    g1 = pwk.tile([S, H], F16, tag="g1")
                    nc.vector.scalar_tensor_tensor(
                        out=g1, in0=g1p, scalar=rec_sb[:, c1 : c1 + 1],
                        in1=bch_sb, op0=mult, op1=add,
                    )
                    T_g1[c1] = g1

                # ---- h_s(c3): transposed weighted sums, copy, store
                if 0 <= c3 < C:
                    gb3, aat3 = T_aat.pop(c3)
                    x16_3 = T_x16.pop(c3)
                    hsp = gb3[:, TC : TC + 2 * S]
                    for h in range(2):
                        for k in range(KT):
                            nc.tensor.matmul(
                                hsp[:, h * S : (h + 1) * S],
                                x16_3[:, k, h * 128 : (h + 1) * 128],
                                aat3[:, k, :],
                                start=(k == 0), stop=(k == KT - 1),
                            )
                    if c3 % GST == 0:
                        T_hsg[c3 // GST] = pwk.tile([128, GST, 2 * S], F32, tag="hsg",
                                                  name="hsg")
                    hsg = T_hsg[c3 // GST]
                    nc.vector.tensor_copy(out=hsg[:, c3 % GST], in_=hsp)
                    if c3 % GST == GST - 1 or c3 == C - 1:
                        n = c3 % GST + 1
                        lo = c3 - n + 1
                        nc.gpsimd.dma_start(
                            out=hst[:, lo : c3 + 1, :], in_=hsg[:, :n],
                        )
                        del T_hsg[c3 // GST]

    _split_multi_waits(nc)
    return nc


_CACHE = {}


def kernel(hidden, pos_table, W_pos, b_pos, W1, b1, W2, b2, qw, qb,
           seq_len, reverse_pos):
    seq_len_np = np.asarray(seq_len)
    lens, core_chunks, C = _plan(seq_len_np)
    weights, qbp, pw8 = _pack_weights(
        pos_table, W_pos, b_pos, W1, b1, W2, b2, qw, qb
    )
    x16, xt8, pf8, seg_row, seg_col, recip, unpack_idx = _pack_inputs(
        hidden, reverse_pos, pw8, lens, core_chunks, C
    )

    key = (C, qbp)
    if key not in _CACHE:
        _CACHE[key] = _build_bass(C, qbp)
    nc = _CACHE[key]

    CKT = C * KT
    W32 = CKT + H + 1 + C
    in_maps = []
    for core in range(N_CORES):
        cf32 = np.zeros((128, W32), np.float32)
        cf32[:, :CKT] = seg_col[core].reshape(128, CKT)
        cf32[:S, CKT : CKT + H] = weights["bchrow"]
        cf32[:S, CKT + H] = np.arange(S, dtype=np.float32)
        cf32[:S, CKT + H + 1 :] = recip[core]
        m = dict(
            x16=x16[core], xt8=xt8[core], pf8=pf8[core],
            seg_row=seg_row[core], cf32=cf32,
            wk8=weights["wk8"], cf16=weights["cf16"],
        )
        in_maps.append(m)

    import time as _time

    t0 = _time.perf_counter()
    res = run_bass_kernel_spmd(nc, in_maps, core_ids=list(range(N_CORES)))
    kernel._last_run_s = _time.perf_counter() - t0
    # hst: [N, 128, C, 2S] f32 -> h_s[sess, h] with h = 128*half + p
    hs_all = np.stack([res.results[i]["hst"] for i in range(N_CORES)])
    hs_all = hs_all.reshape(N_CORES, 128, C, 2, S)

    out_core, out_chunk, out_local = unpack_idx
    # [sess, half, p] -> [sess, 128*half + p]
    out = hs_all[out_core, :, out_chunk, :, out_local]      # [B, 128, 2]
    out = out.transpose(0, 2, 1).reshape(len(out_core), H)
    return np.ascontiguousarray(out)


# revision 37
# speedup vs baseline: 2.5840x; 1.0012x over previous
"""Trainium2 Bass kernel for the CNNFusing ragged-session attention pooling module.

Computes, per session s over its token set:
    v_mean   = mean(hidden[s])                                  [H]
    ph[t]    = tanh(hidden[t] @ Wp1 + (pos_table @ Wp2 + b_pos)[rp[t]])
    gate[t]  = sigmoid(v_mean @ W1 + b1 + ph[t] @ W2 + b2)
    alpha[t] = gate[t] @ qw + qb
    h_s      = sum_t alpha[t] * hidden[t]                       [B, H]

Data-parallel over sessions on 8 cores.  Sessions are FFD-bin-packed into
512-token chunks (<=32 sessions per chunk).  Ragged ops become one-hot
matmuls.  The two big H x H GEMM chains per token (ph pre-act and the gate
ph-term) run as fp8e4m3 DoubleRow matmuls (2 contraction rows/partition);
session sums / weighted sums stay fp16.  h_s is accumulated transposed
([h, s] layout) so each accumulation step costs S output rows instead of H.
The sigmoid is folded into tanh and both per-token biases ride the one-hot
mean-term matmul, so each stage needs a single full-width tanh activation.
"""

import numpy as np
import ml_dtypes

import concourse.bass as bass
import concourse.mybir as mybir
import concourse.tile as tile
from concourse.bass_utils import run_bass_kernel_spmd

H = 256
TC = 512      # tokens per chunk
S = 32        # max sessions per chunk
KT = TC // 128
G = 2         # chunks per batched load DMA
SG = 8      # chunks per seg-row broadcast DMA
GST = 8     # chunks per batched store
N_CORES = 8

F8 = mybir.dt.float8e4
F16 = mybir.dt.float16
F32 = mybir.dt.float32
NP_F8 = ml_dtypes.float8_e4m3fn


# --------------------------------------------------------------------------
# The walrus build here accepts only ONE sync-wait command per instruction,
# while Tile may attach several.  Hoist all but the last wait of such
# instructions onto standalone event-semaphore waits inserted just before
# them on the same engine (sequencer executes in order, semantics kept).
_waitsplit_uid = [0]


def _split_multi_waits(nc):
    for fn in nc.m.functions:
        for bb in fn.blocks:
            insts = bb.instructions
            i = 0
            while i < len(insts):
                inst = insts[i]
                si = getattr(inst, "sync_info", None)
                waits = list(si.on_wait) if si is not None and si.on_wait else []
                if len(waits) > 1:
                    si.on_wait = waits[-1:]
                    for w in waits[:-1]:
                        ev = mybir.InstEventSemaphore(
                            name=f"I-waitsplit-{_waitsplit_uid[0]}", ins=[], outs=[]
                        )
                        _waitsplit_uid[0] += 1
                        ev.engine = inst.engine
                        ev.sync_info = mybir.SyncInfo(on_wait=[w], on_update=[])
                        insts.insert(i, ev)
                        i += 1
                i += 1
# --------------------------------------------------------------------------


def _plan(seq_len):
    """Best-fit-decreasing bin packing of all sessions into (token<=TC,
    sessions<=S) chunks, then deal chunks round-robin to cores."""
    lens = np.asarray(seq_len, dtype=np.int64)
    B = len(lens)
    order = np.argsort(-lens, kind="stable")
    bins = []          # list of [tok_used, [session ids]]
    # rem_sorted: sorted list of (remaining_tokens, bin_idx) for best-fit
    import bisect
    rem = []           # sorted (remaining, bin_idx)
    for sid in order:
        L = int(lens[sid])
        # best fit: smallest remaining >= L
        pos = bisect.bisect_left(rem, (L, -1))
        placed = False
        while pos < len(rem):
            r, bi = rem[pos]
            if len(bins[bi][1]) < S:
                rem.pop(pos)
                bins[bi][0] += L
                bins[bi][1].append(sid)
                nr = TC - bins[bi][0]
                if nr > 0:
                    bisect.insort(rem, (nr, bi))
                placed = True
                break
            pos += 1
        if not placed:
            bi = len(bins)
            bins.append([L, [sid]])
            bisect.insort(rem, (TC - L, bi))
    nb = len(bins)
    C = -(-nb // N_CORES)
    core_chunks = [[] for _ in range(N_CORES)]
    for i, b in enumerate(bins):
        core_chunks[i % N_CORES].append(b[1])
    return lens, core_chunks, C


def _pack_inputs(hidden, reverse_pos, pw8, lens, core_chunks, C):
    """Build all per-core DRAM input arrays."""
    B = len(lens)
    starts = np.concatenate([[0], np.cumsum(lens)[:-1]])
    hidden16 = np.asarray(hidden, np.float32).astype(np.float16)
    rp = np.asarray(reverse_pos)

    # token map [N, C, TC] -> global token index (or -1)
    tokmap = np.full((N_CORES, C, TC), -1, np.int64)
    seg_row = np.full((N_CORES, C, TC), -1.0, np.float16)
    recip = np.zeros((N_CORES, S, C), np.float32)
    out_core = np.zeros(B, np.int32)
    out_chunk = np.zeros(B, np.int32)
    out_local = np.zeros(B, np.int32)

    for core in range(N_CORES):
        for ci, sess in enumerate(core_chunks[core]):
            t = 0
            for si, sid in enumerate(sess):
                L = int(lens[sid])
                tokmap[core, ci, t : t + L] = np.arange(starts[sid], starts[sid] + L)
                seg_row[core, ci, t : t + L] = si
                recip[core, si, ci] = 1.0 / L
                out_core[sid] = core
                out_chunk[sid] = ci
                out_local[sid] = si
                t += L

    valid = tokmap >= 0
    idx = np.where(valid, tokmap, 0)

    # gathered hidden [N, C, TC, H] fp16 (zero padded)
    xt = hidden16[idx]
    xt[~valid] = 0
    # row tiles [N, C, 128, KT, H] f16
    x16 = np.ascontiguousarray(
        xt.reshape(N_CORES, C, KT, 128, H).transpose(0, 1, 3, 2, 4)
    )
    # transposed fp8 [N, C, 128, 2, TC]
    xt8 = np.ascontiguousarray(
        xt.astype(NP_F8).transpose(0, 1, 3, 2).reshape(N_CORES, C, 2, 128, TC)
        .transpose(0, 1, 3, 2, 4)
    )
    del xt

    # pos features (already fp8-quantized table), gathered transposed
    rpg = np.where(valid, rp[idx], 0)
    pft = pw8[rpg]                                  # [N, C, TC, H] fp8
    pft[~valid] = 0
    pf8 = np.ascontiguousarray(
        pft.transpose(0, 1, 3, 2).reshape(N_CORES, C, 2, 128, TC)
        .transpose(0, 1, 3, 2, 4)
    )
    del pft

    seg_col = np.ascontiguousarray(
        seg_row.reshape(N_CORES, C, KT, 128).transpose(0, 3, 1, 2)
    ).astype(np.float32)

    return x16, xt8, pf8, seg_row, seg_col, recip, (out_core, out_chunk, out_local)


def _pack_weights(pos_table, W_pos, b_pos, W1, b1, W2, b2, qw, qb):
    Wp = np.asarray(W_pos, np.float32)
    pwf = np.asarray(pos_table, np.float32) @ Wp[H:] + np.asarray(b_pos, np.float32)
    pw8 = np.zeros((H, H), NP_F8)
    pw8[: pwf.shape[0]] = pwf.astype(NP_F8)

    def pack_dr(M):  # [256, 256] -> [128, 2, 256] fp8, row c = 128*i + p
        return np.ascontiguousarray(
            np.asarray(M, np.float32).reshape(2, 128, H).transpose(1, 0, 2)
        ).astype(NP_F8)

    wp18 = pack_dr(Wp[:H])
    w28 = pack_dr(np.asarray(W2, np.float32))
    w18 = pack_dr(np.asarray(W1, np.float32))

    ident8 = np.zeros((128, 2, H), NP_F8)
    for m in range(2):
        ident8[:, m, m * 128 : (m + 1) * 128] = np.eye(128, dtype=NP_F8)

    qwf = np.asarray(qw, np.float32).reshape(H)
    qwh = np.ascontiguousarray(qwf.reshape(2, 128).T).astype(np.float16)
    qbp = float(np.asarray(qb, np.float32).reshape(()) + qwf.sum() / 2.0)
    # full (unscaled) bias b1+b2 rides the mean-term; ACT applies tanh(z/2)
    bcf = np.asarray(b1, np.float32) + np.asarray(b2, np.float32)
    bchrow = np.broadcast_to(bcf, (S, H)).copy().astype(np.float32)

    iota_at = np.broadcast_to(np.arange(S, dtype=np.float16), (128, S)).copy()
    wk8 = np.concatenate([wp18, ident8, w28, w18], axis=2)
    cf16 = np.concatenate([qwh, iota_at], axis=1).astype(np.float16)
    return dict(wk8=wk8, cf16=cf16, bchrow=bchrow), qbp, pw8


def _build_bass(C, qbp):
    nc = bass.Bass("TRN2", target_bir_lowering=False, debug=False,
                   num_devices=N_CORES)

    x16 = nc.dram_tensor("x16", [C, 128, KT, H], F16, kind="ExternalInput")
    xt8 = nc.dram_tensor("xt8", [C, 128, 2, TC], F8, kind="ExternalInput")
    pf8 = nc.dram_tensor("pf8", [C, 128, 2, TC], F8, kind="ExternalInput")
    seg_row = nc.dram_tensor("seg_row", [C, TC], F16, kind="ExternalInput")
    W32 = C * KT + H + 1 + C
    wk8 = nc.dram_tensor("wk8", [128, 2, 4 * H], F8, kind="ExternalInput")
    cf32 = nc.dram_tensor("cf32", [128, W32], F32, kind="ExternalInput")
    cf16 = nc.dram_tensor("cf16", [128, 2 + S], F16, kind="ExternalInput")
    hst = nc.dram_tensor("hst", [128, C, 2 * S], F32, kind="ExternalOutput")

    eq = mybir.AluOpType.is_equal
    mult = mybir.AluOpType.mult
    add = mybir.AluOpType.add
    Tanh = mybir.ActivationFunctionType.Tanh
    DR = mybir.MatmulPerfMode.DoubleRow

    NG = -(-C // G)    # number of load groups

    with tile.TileContext(nc) as tc:
        with (
            tc.tile_pool(name="consts", bufs=1) as pc,
            tc.tile_pool(name="loads", bufs=6) as pl,
            tc.tile_pool(name="segp", bufs=3) as psg,
            tc.tile_pool(name="work", bufs=10) as pwk,
            # PSUM: ph 1x2 banks + ga 2x1 + gate 2x2 = 8 banks
            tc.tile_pool(name="pph", bufs=1, space="PSUM") as pph,
            tc.tile_pool(name="pga", bufs=2, space="PSUM") as pga,
            tc.tile_pool(name="pgt", bufs=2, space="PSUM") as pgt,
        ):
            # ---- constants: 3 packed DMAs keep startup short ----
            wk8_sb = pc.tile([128, 2, 4 * H], F8)
            nc.sync.dma_start(out=wk8_sb, in_=wk8[:])
            cf16_sb = pc.tile([128, 2 + S], F16)
            cf32_sb = pc.tile([128, W32], F32)
            wp18_sb = wk8_sb[:, :, 0 * H : 1 * H]
            id8_sb = wk8_sb[:, :, 1 * H : 2 * H]
            w28_sb = wk8_sb[:, :, 2 * H : 3 * H]
            w18_sb = wk8_sb[:, :, 3 * H : 4 * H]
            qwh_sb = cf16_sb[:, 0:2]
            iota_at_sb = cf16_sb[:, 2 : 2 + S]
            segc_sb = cf32_sb[:, 0 : C * KT].rearrange("p (c k) -> p c k", c=C)
            bch_sb = cf32_sb[0:S, C * KT : C * KT + H]
            iota_s_sb = cf32_sb[0:S, C * KT + H : C * KT + H + 1]
            rec_sb = cf32_sb[0:S, C * KT + H + 1 : C * KT + H + 1 + C]

            T_x16, T_xt8, T_pf8 = {}, {}, {}
            T_segb = {}
            T_as, T_at, T_ph8, T_g1, T_smt, T_aat = (
                {}, {}, {}, {}, {}, {}
            )
            T_hsg = {}

            def emit_loads(g, part=None):
                c = g * G
                ng = min(G, C - c)
                if part in (None, 0):
                    xt8_t = pl.tile([128, G, 2, TC], F8, tag="xt8")
                    nc.sync.dma_start(
                        out=xt8_t[:, :ng],
                        in_=xt8[c : c + ng].rearrange("c p i t -> p c i t"),
                    )
                    pf8_t = pl.tile([128, G, 2, TC], F8, tag="pf8")
                    nc.sync.dma_start(
                        out=pf8_t[:, :ng],
                        in_=pf8[c : c + ng].rearrange("c p i t -> p c i t"),
                    )
                    for j in range(ng):
                        T_xt8[c + j] = xt8_t[:, j]
                        T_pf8[c + j] = pf8_t[:, j]
                if part in (None, 1):
                    x16_t = pl.tile([128, G, KT, H], F16, tag="x16")
                    nc.sync.dma_start(
                        out=x16_t[:, :ng],
                        in_=x16[c : c + ng].rearrange("c p k h -> p c k h"),
                    )
                    for j in range(ng):
                        T_x16[c + j] = x16_t[:, j]

            def emit_seg(sg):
                c = sg * SG
                n = min(SG, C - c)
                src = seg_row[c]
                segb = psg.tile([S, SG * TC], F16, tag="segb")
                nc.sync.dma_start(
                    out=segb[:, : n * TC],
                    in_=bass.AP(tensor=src.tensor, offset=src.offset,
                                ap=[[0, S], [1, n * TC]]),
                )
                T_segb[sg] = segb

            emit_loads(0, part=0)
            nc.sync.dma_start(out=cf16_sb, in_=cf16[:])
            nc.sync.dma_start(out=cf32_sb, in_=cf32[:])
            emit_seg(0)
            emit_loads(0, part=1)
            if NG > 1:
                emit_loads(1)

            for it in range(C + 3):
                c0, c1, c2, c3 = it, it - 1, it - 2, it - 3

                # prefetch
                if c0 % G == 0 and c0 // G + 2 < NG:
                    emit_loads(c0 // G + 2)
                if c0 % SG == 0 and c0 // SG + 1 <= (C - 1) // SG:
                    emit_seg(c0 // SG + 1)

                # ---- masks(c0): first on DVE (deps always ready)
                if c0 < C:
                    sg, si = c0 // SG, c0 % SG
                    segb = T_segb[sg]
                    a_s = pwk.tile([S, TC], F16, tag="a_s")
                    nc.vector.tensor_single_scalar(
                        out=a_s, in_=segb[:, si * TC : (si + 1) * TC],
                        scalar=iota_s_sb, op=eq,
                    )
                    a_t = pwk.tile([128, KT, S], F16, tag="a_t")
                    nc.vector.tensor_tensor(
                        out=a_t,
                        in0=bass.AP(tensor=iota_at_sb.tensor, offset=iota_at_sb.offset,
                                    ap=[list(iota_at_sb.ap[0]), [0, KT], [1, S]]),
                        in1=bass.AP(tensor=segc_sb.tensor,
                                    offset=segc_sb.offset + c0 * KT,
                                    ap=[list(segc_sb.ap[0]), [1, KT], [0, S]]),
                        op=eq,
                    )
                    T_as[c0] = a_s
                    T_at[c0] = a_t

                # ---- gate(c2) matmuls + tanh
                if 0 <= c2 < C:
                    g1_2 = T_g1.pop(c2)
                    a_s2 = T_as.pop(c2)
                    ph8_2 = T_ph8.pop(c2)
                    gp = pgt.tile([128, 2 * TC], F32, tag="gate")
                    for h in range(2):
                        dst = gp[:, h * TC : (h + 1) * TC]
                        nc.tensor.matmul(
                            dst, g1_2[:, h * 128 : (h + 1) * 128], a_s2,
                            start=True, stop=False,
                        )
                        nc.tensor.matmul(
                            dst,
                            w28_sb[:, :, h * 128 : (h + 1) * 128],
                            ph8_2.rearrange("p (i t) -> p i t", i=2),
                            start=False, stop=True, perf_mode=DR,
                        )
                    gt2 = pwk.tile([128, 2 * TC], F16, tag="gt")
                    nc.scalar.activation(out=gt2, in_=gp, func=Tanh, scale=0.5)
                    alp = gp[:, 0:KT]
                    for k in range(KT):
                        for h in range(2):
                            nc.tensor.matmul(
                                alp[:, k : k + 1],
                                gt2[:, h * TC + k * 128 : h * TC + (k + 1) * 128],
                                qwh_sb[:, h : h + 1],
                                start=(h == 0), stop=(h == 1),
                            )
                    alpha = pwk.tile([128, KT], F32, tag="alpha")
                    nc.vector.tensor_scalar(
                        out=alpha, in0=alp, scalar1=0.5, scalar2=qbp,
                        op0=mult, op1=add,
                    )
                    a_t2 = T_at.pop(c2)
                    aat = pwk.tile([128, KT, S], F16, tag="aat")
                    nc.vector.tensor_tensor(
                        out=aat,
                        in0=a_t2,
                        in1=bass.AP(tensor=alpha.tensor, offset=alpha.offset,
                                    ap=[list(alpha.ap[0]), [1, KT], [0, S]]),
                        op=mult,
                    )
                    T_aat[c2] = (gp, aat)

                # ---- ph(c0): DR matmuls + tanh -> fp8
                if c0 < C:
                    xt8_0 = T_xt8.pop(c0)
                    pf8_0 = T_pf8.pop(c0)
                    php = pph.tile([128, 2 * TC], F32, tag="ph")
                    for h in range(2):
                        dst = php[:, h * TC : (h + 1) * TC]
                        nc.tensor.matmul(
                            dst, wp18_sb[:, :, h * 128 : (h + 1) * 128], xt8_0,
                            start=True, stop=False, perf_mode=DR,
                        )
                        nc.tensor.matmul(
                            dst, id8_sb[:, :, h * 128 : (h + 1) * 128], pf8_0,
                            start=False, stop=True, perf_mode=DR,
                        )
                    ph8 = pwk.tile([128, 2 * TC], F8, tag="ph8")
                    nc.scalar.activation(out=ph8, in_=php, func=Tanh)
                    T_ph8[c0] = ph8

                # ---- ss(c0): transposed session sums + fp8 copy
                if c0 < C:
                    x16_0 = T_x16[c0]
                    a_t0 = T_at[c0]
                    ga = pga.tile([128, 2 * S + H], F32, tag="ga")
                    ss = ga[:, 0 : 2 * S]
                    for h in range(2):
                        for k in range(KT):
                            nc.tensor.matmul(
                                ss[:, h * S : (h + 1) * S],
                                x16_0[:, k, h * 128 : (h + 1) * 128],
                                a_t0[:, k, :],
                                start=(k == 0), stop=(k == KT - 1),
                            )
                    smt = pwk.tile([128, 2 * S], F8, tag="smt")
                    nc.vector.tensor_copy(out=smt, in_=ss)
                    T_smt[c0] = (smt, ga)

                # ---- g1(c1): DR matmul + scale/bias (late on PE so the
                # smt copy from last iteration has fully drained)
                if 0 <= c1 < C:
                    smt1, ga1 = T_smt.pop(c1)
                    g1p = ga1[0:S, 2 * S : 2 * S + H]
                    nc.tensor.matmul(
                        g1p,
                        smt1.rearrange("p (i s) -> p i s", i=2),
                        w18_sb[:],
                        start=True, stop=True, perf_mode=DR,
                    )
                    g1 = pwk.tile([S, H], F16, tag="g1")
                    nc.vector.scalar_tensor_tensor(
                        out=g1, in0=g1p, scalar=rec_sb[:, c1 : c1 + 1],
                        in1=bch_sb, op0=mult, op1=add,
                    )
                    T_g1[c1] = g1

                # ---- h_s(c3): transposed weighted sums, copy, store
                if 0 <= c3 < C:
                    gb3, aat3 = T_aat.pop(c3)
                    x16_3 = T_x16.pop(c3)
                    hsp = gb3[:, TC : TC + 2 * S]
                    for h in range(2):
                        for k in range(KT):
                            nc.tensor.matmul(
                                hsp[:, h * S : (h + 1) * S],
                                x16_3[:, k, h * 128 : (h + 1) * 128],
                                aat3[:, k, :],
                                start=(k == 0), stop=(k == KT - 1),
                            )
                    grp = c3 // GST
                    if c3 % GST == 0:
                        T_hsg[grp] = [pwk.tile([128, GST, 2 * S], F32, tag="hsg",
                                               name="hsg"), c3]
                    hsg, lo = T_hsg[grp]
                    nc.vector.tensor_copy(out=hsg[:, c3 % GST], in_=hsp)
                    last_grp = grp == (C - 1) // GST
                    end = c3 % GST == GST - 1 or c3 == C - 1
                    if end or (last_grp and c3 % 2 == 1):
                        nc.gpsimd.dma_start(
                            out=hst[:, lo : c3 + 1, :],
                            in_=hsg[:, lo - grp * GST : c3 % GST + 1],
                        )
                        T_hsg[grp][1] = c3 + 1
                        if end:
                            del T_hsg[grp]

    _split_multi_waits(nc)
    return nc


_CACHE = {}


def kernel(hidden, pos_table, W_pos, b_pos, W1, b1, W2, b2, qw, qb,
           seq_len, reverse_pos):
    seq_len_np = np.asarray(seq_len)
    lens, core_chunks, C = _plan(seq_len_np)
    weights, qbp, pw8 = _pack_weights(
        pos_table, W_pos, b_pos, W1, b1, W2, b2, qw, qb
    )
    x16, xt8, pf8, seg_row, seg_col, recip, unpack_idx = _pack_inputs(
        hidden, reverse_pos, pw8, lens, core_chunks, C
    )

    key = (C, qbp)
    if key not in _CACHE:
        _CACHE[key] = _build_bass(C, qbp)
    nc = _CACHE[key]

    CKT = C * KT
    W32 = CKT + H + 1 + C
    in_maps = []
    for core in range(N_CORES):
        cf32 = np.zeros((128, W32), np.float32)
        cf32[:, :CKT] = seg_col[core].reshape(128, CKT)
        cf32[:S, CKT : CKT + H] = weights["bchrow"]
        cf32[:S, CKT + H] = np.arange(S, dtype=np.float32)
        cf32[:S, CKT + H + 1 :] = recip[core]
        m = dict(
            x16=x16[core], xt8=xt8[core], pf8=pf8[core],
            seg_row=seg_row[core], cf32=cf32,
            wk8=weights["wk8"], cf16=weights["cf16"],
        )
        in_maps.append(m)

    import time as _time

    t0 = _time.perf_counter()
    res = run_bass_kernel_spmd(nc, in_maps, core_ids=list(range(N_CORES)))
    kernel._last_run_s = _time.perf_counter() - t0
    # hst: [N, 128, C, 2S] f32 -> h_s[sess, h] with h = 128*half + p
    hs_all = np.stack([res.results[i]["hst"] for i in range(N_CORES)])
    hs_all = hs_all.reshape(N_CORES, 128, C, 2, S)

    out_core, out_chunk, out_local = unpack_idx
    # [sess, half, p] -> [sess, 128*half + p]
    out = hs_all[out_core, :, out_chunk, :, out_local]      # [B, 128, 2]
    out = out.transpose(0, 2, 1).reshape(len(out_core), H)
    return np.ascontiguousarray(out)
